# revision 1
# baseline (speedup 1.0000x reference)
"""Trainium2 Bass kernel for a dense transformer block (B=128, T=256, C=384, H=6).

Sharding: data-parallel over batch across 8 NeuronCores (16 batches/core),
identical SPMD program per core, no collectives.

Per-core schedule: batches in pairs (free dim 512 in the big matmuls).
All matmuls in float32r (fp22-truncated fp32 at full PE speed for free
dim >= 256, fp32 PSUM accumulate).

This container's ACT piecewise-poly tables are broken (any table-based
activation crashes the device), so all transcendentals are built from
table-free DVE ops:
  - exp: Schraudolph bit-trick + quadratic mantissa correction (~3.5e-3 rel)
  - rsqrt (layernorm): sqrt bit-trick + 2 Heron steps + native reciprocal
  - relu: tensor_scalar (add bias, max 0)
Softmax skips max-subtraction (scores are O(1) here); the causal mask is
multiplicative post-exp; row-sum fuses into the mask multiply
(tensor_tensor_reduce); 1/rowsum folds into the PE transpose of the
attention weights via a diagonal rhs operand.
"""

import numpy as np

import concourse.bass as bass
import concourse.mybir as mybir
from concourse import bacc
from concourse.tile import TileContext
from contextlib import ExitStack

B, T, C = 128, 256, 384
H, D = 6, 64
FF = 4 * C
NCORES = 8
BL = B // NCORES  # 16
NPAIR = BL // 2  # 8
KC = C // 128  # 3
KH = FF // 128  # 12
EPS = 1e-5
F32 = mybir.dt.float32
F32R = mybir.dt.float32r
I32 = mybir.dt.int32
ALU = mybir.AluOpType

# exp = Schraudolph + quadratic mantissa correction (validated on HW: 3.5e-3)
EXP_S = float(2**23 / np.log(2.0))
EXP_B = float(127 * 2**23)
_C2, _C1, _C0 = 0.23374667, -0.2270202, 0.99663616
_ALPHA = _C1 / (2 * _C2)
_BETA = _C0 / _C2 - _ALPHA * _ALPHA
U_SCALE = float(np.sqrt(_C2) / 2**23)
U_BIAS = float(_ALPHA * np.sqrt(_C2))
E_BIAS = float(_BETA * _C2)
SQRT_MAGIC = 0x1FBD1DF5


def build_program(use_g1, use_b1ln, use_g2, use_b2ln, use_bp, use_b1, use_b2):
    nc = bacc.Bacc(None)
    x = nc.declare_dram_parameter("x", [BL, T, C], F32, isOutput=False)
    wq = nc.declare_dram_parameter("wq", [C, C], F32R, isOutput=False)
    wk = nc.declare_dram_parameter("wk", [C, C], F32R, isOutput=False)
    wv = nc.declare_dram_parameter("wv", [C, C], F32R, isOutput=False)
    wp = nc.declare_dram_parameter("wp", [C, C], F32R, isOutput=False)
    w1 = nc.declare_dram_parameter("w1", [C, FF], F32R, isOutput=False)
    w2 = nc.declare_dram_parameter("w2", [FF, C], F32R, isOutput=False)
    g1 = nc.declare_dram_parameter("g1", [128, C], F32, isOutput=False)
    b1ln = nc.declare_dram_parameter("b1ln", [128, C], F32, isOutput=False)
    g2 = nc.declare_dram_parameter("g2", [128, C], F32, isOutput=False)
    b2ln = nc.declare_dram_parameter("b2ln", [128, C], F32, isOutput=False)
    bpb = nc.declare_dram_parameter("bpb", [128, C], F32, isOutput=False)
    b2b = nc.declare_dram_parameter("b2b", [128, C], F32, isOutput=False)
    b1c = nc.declare_dram_parameter("b1c", [128, KH], F32, isOutput=False)
    m0 = nc.declare_dram_parameter("m0", [128, T], F32, isOutput=False)
    m1m = nc.declare_dram_parameter("m1m", [128, T], F32, isOutput=False)
    ident = nc.declare_dram_parameter("ident", [128, 128], F32, isOutput=False)
    out = nc.declare_dram_parameter("out", [BL, T, C], F32, isOutput=True)

    with TileContext(nc) as tc, ExitStack() as ctx:
        wts = ctx.enter_context(tc.tile_pool(name="wts", bufs=1))
        sb = ctx.enter_context(tc.tile_pool(name="sb", bufs=1))
        st = ctx.enter_context(tc.tile_pool(name="st", bufs=4))
        tr = ctx.enter_context(tc.tile_pool(name="tr", bufs=4))
        ps = ctx.enter_context(tc.tile_pool(name="ps", bufs=4, space="PSUM"))
        psy = ctx.enter_context(tc.tile_pool(name="psy", bufs=1, space="PSUM"))

        def load_chunks(dram, n, width, tagp):
            tiles = []
            for k in range(n):
                t_ = wts.tile(
                    [128, width], F32R, name=f"{tagp}{k}", tag=f"{tagp}{k}"
                )
                nc.sync.dma_start(out=t_, in_=dram[k * 128 : (k + 1) * 128, :])
                tiles.append(t_)
            return tiles

        wq_sb = load_chunks(wq, KC, C, "wq")
        wk_sb = load_chunks(wk, KC, C, "wk")
        wv_sb = load_chunks(wv, KC, C, "wv")
        wp_sb = load_chunks(wp, KC, C, "wp")
        w1_sb = load_chunks(w1, KC, FF, "w1")
        w2_sb = load_chunks(w2, KH, C, "w2")

        def load_one(dram, shape, tag):
            t_ = wts.tile(shape, F32, name=tag, tag=tag)
            nc.sync.dma_start(out=t_, in_=dram[:, :])
            return t_

        g1_sb = load_one(g1, [128, C], "g1") if use_g1 else None
        b1ln_sb = load_one(b1ln, [128, C], "b1ln") if use_b1ln else None
        g2_sb = load_one(g2, [128, C], "g2") if use_g2 else None
        b2ln_sb = load_one(b2ln, [128, C], "b2ln") if use_b2ln else None
        bpb_sb = load_one(bpb, [128, C], "bpb") if use_bp else None
        b2b_sb = load_one(b2b, [128, C], "b2b") if use_b2 else None
        b1c_sb = load_one(b1c, [128, KH], "b1c") if use_b1 else None
        m0_sb = load_one(m0, [128, T], "m0")
        m1_sb = load_one(m1m, [128, T], "m1m")
        id_sb = load_one(ident, [128, 128], "ident")

        for p in range(NPAIR):
            bs = [2 * p, 2 * p, 2 * p + 1, 2 * p + 1]
            tch = [0, 1, 0, 1]

            def batched_rstd(mv8):
                """[128,8] interleaved (mean,var) x4 -> rstd4 [128,4]."""
                mv_v = mv8.rearrange("p (i two) -> p i two", two=2)
                var4 = mv_v[:, :, 1]
                vpe = st.tile([128, 4], F32, name="vpe", tag="vpe")
                nc.vector.tensor_scalar(
                    out=vpe, in0=var4, scalar1=EPS, scalar2=None, op0=ALU.add
                )
                s0h = st.tile([128, 4], I32, name="s0h", tag="s0h")
                nc.vector.tensor_scalar(
                    out=s0h, in0=vpe.bitcast(I32), scalar1=1, scalar2=None,
                    op0=ALU.logical_shift_right,
                )
                s0i = st.tile([128, 4], I32, name="s0i", tag="s0i")
                nc.vector.tensor_scalar(
                    out=s0i, in0=s0h, scalar1=SQRT_MAGIC, scalar2=None,
                    op0=ALU.add,
                )
                cur = s0i.bitcast(F32)
                for hi in range(2):
                    r_ = st.tile([128, 4], F32, name=f"hr{hi}", tag=f"hr{hi}")
                    nc.vector.reciprocal(r_, cur)
                    t_ = st.tile([128, 4], F32, name=f"ht{hi}", tag=f"ht{hi}")
                    nc.vector.tensor_mul(t_, vpe, r_)
                    s_ = st.tile([128, 4], F32, name=f"hs{hi}", tag=f"hs{hi}")
                    nc.vector.tensor_add(s_, t_, cur)
                    sh = st.tile([128, 4], F32, name=f"hh{hi}", tag=f"hh{hi}")
                    nc.vector.tensor_scalar_mul(sh, s_, 0.5)
                    cur = sh
                rstd4 = st.tile([128, 4], F32, name="rstd4", tag="rstd4")
                nc.vector.reciprocal(rstd4, cur)
                return rstd4

            def layernorm4(dsts, srcs, g_sb, b_sb):
                mv8 = st.tile([128, 8], F32, name="mv8", tag="mv8")
                for i in range(4):
                    stats = st.tile([128, 6], F32, name="lst", tag="lst")
                    nc.vector.bn_stats(stats, srcs[i])
                    nc.vector.bn_aggr(mv8[:, 2 * i : 2 * i + 2], stats)
                rstd4 = batched_rstd(mv8)
                for i in range(4):
                    nc.vector.tensor_scalar(
                        out=dsts[i], in0=srcs[i],
                        scalar1=mv8[:, 2 * i : 2 * i + 1],
                        scalar2=rstd4[:, i : i + 1],
                        op0=ALU.subtract, op1=ALU.mult,
                    )
                    if g_sb is not None:
                        nc.vector.tensor_mul(dsts[i], dsts[i], g_sb)
                    if b_sb is not None:
                        nc.vector.tensor_add(dsts[i], dsts[i], b_sb)

            def transpose_into(dstT, src, i):
                for c in range(KC):
                    pt = ps.tile([128, 128], F32, name="pa", tag="pa")
                    nc.tensor.transpose(
                        pt, src[:, c * 128 : (c + 1) * 128], id_sb
                    )
                    nc.scalar.copy(dstT[c][:, i * 128 : (i + 1) * 128], pt)

            # ---- stage 1: load x, LN1, transpose -> hT ----
            xt = [
                sb.tile([128, C], F32, name=f"xt{i}", tag=f"xt{i}", bufs=2)
                for i in range(4)
            ]
            for i in range(4):
                nc.sync.dma_start(
                    out=xt[i],
                    in_=x[bs[i], tch[i] * 128 : (tch[i] + 1) * 128, :],
                )
            hT = [
                sb.tile([128, 2 * T], F32R, name=f"hT{c}", tag=f"hT{c}", bufs=2)
                for c in range(KC)
            ]
            ht_ = [
                sb.tile([128, C], F32, name=f"h{i}", tag=f"h{i}")
                for i in range(4)
            ]
            layernorm4(ht_, xt, g1_sb, b1ln_sb)
            for i in range(4):
                transpose_into(hT, ht_[i], i)

            # ---- stage 2: q^T, k^T (C-major), v (token-major) ----
            qT = [
                sb.tile([128, 2 * T], F32R, name=f"qT{m}", tag=f"qT{m}")
                for m in range(KC)
            ]
            kT = [
                sb.tile([128, 2 * T], F32R, name=f"kT{m}", tag=f"kT{m}")
                for m in range(KC)
            ]
            for m in range(KC):
                pq = ps.tile([128, 2 * T], F32, name="pa", tag="pa")
                for k in range(KC):
                    nc.tensor.matmul(
                        pq, wq_sb[k][:, m * 128 : (m + 1) * 128], hT[k],
                        start=(k == 0), stop=(k == KC - 1),
                    )
                nc.scalar.copy(qT[m], pq)
                pk = ps.tile([128, 2 * T], F32, name="pa", tag="pa")
                for k in range(KC):
                    nc.tensor.matmul(
                        pk, wk_sb[k][:, m * 128 : (m + 1) * 128], hT[k],
                        start=(k == 0), stop=(k == KC - 1),
                    )
                nc.scalar.copy(kT[m], pk)
            vt = [
                sb.tile([128, C], F32R, name=f"v{i}", tag=f"v{i}")
                for i in range(4)
            ]
            for i in range(4):
                pv = ps.tile([128, C], F32, name="pa", tag="pa")
                for k in range(KC):
                    nc.tensor.matmul(
                        pv, hT[k][:, i * 128 : (i + 1) * 128], wv_sb[k],
                        start=(k == 0), stop=(k == KC - 1),
                    )
                nc.scalar.copy(vt[i], pv)

            # ---- stage 3: attention (head pairs packed into PE col groups) ----
            acT = [
                sb.tile([128, 2 * T], F32R, name=f"acT{c}", tag=f"acT{c}")
                for c in range(KC)
            ]
            for ib in range(2):
                for ch in range(KC):
                    wTs = {}
                    for par in range(2):  # even/odd head of this chunk
                        hh = 2 * ch + par
                        off = par * 64
                        wTs[par] = [
                            tr.tile(
                                [128, T], F32R,
                                name=f"wT{sc}{par}", tag=f"wT{sc}{par}", bufs=2,
                            )
                            for sc in range(2)
                        ]
                        for tc_ in range(2):
                            w_ = 128 if tc_ == 0 else T
                            pS = ps.tile([128, T], F32, name="pa", tag="pa")
                            lhs = qT[ch][
                                off : off + 64,
                                ib * T + tc_ * 128 : ib * T + (tc_ + 1) * 128,
                            ]
                            rhs = kT[ch][off : off + 64, ib * T : (ib + 1) * T]
                            nc.tensor.matmul(pS, lhs, rhs, start=True, stop=True)
                            # exp via Schraudolph + quadratic correction (DVE)
                            it = tr.tile([128, T], I32, name="eit", tag="eit")
                            nc.vector.tensor_scalar(
                                out=it[:, 0:w_], in0=pS[:, 0:w_], scalar1=EXP_S,
                                scalar2=EXP_B, op0=ALU.mult, op1=ALU.add,
                            )
                            mt = tr.tile([128, T], I32, name="emt", tag="emt")
                            nc.vector.tensor_scalar(
                                out=mt[:, 0:w_], in0=it[:, 0:w_],
                                scalar1=0x7FFFFF, scalar2=None,
                                op0=ALU.bitwise_and,
                            )
                            ut = tr.tile([128, T], F32, name="eut", tag="eut")
                            nc.vector.tensor_scalar(
                                out=ut[:, 0:w_], in0=mt[:, 0:w_], scalar1=U_SCALE,
                                scalar2=U_BIAS, op0=ALU.mult, op1=ALU.add,
                            )
                            u2 = tr.tile([128, T], F32, name="eu2", tag="eu2")
                            nc.vector.tensor_mul(
                                u2[:, 0:w_], ut[:, 0:w_], ut[:, 0:w_]
                            )
                            eS = tr.tile([128, T], F32, name="eS", tag="eS")
                            nc.vector.scalar_tensor_tensor(
                                out=eS[:, 0:w_], in0=u2[:, 0:w_], scalar=E_BIAS,
                                in1=it.bitcast(F32)[:, 0:w_],
                                op0=ALU.add, op1=ALU.mult,
                            )
                            wU = tr.tile([128, T], F32, name="wU", tag="wU")
                            rsum = st.tile([128, 1], F32, name="rsum", tag="rsum")
                            nc.vector.scalar_tensor_tensor(
                                out=wU[:, 0:w_], in0=eS[:, 0:w_], scalar=1.0,
                                in1=(m0_sb if tc_ == 0 else m1_sb)[:, 0:w_],
                                op0=ALU.mult, op1=ALU.mult, accum_out=rsum,
                            )
                            rr = st.tile([128, 1], F32, name="rr", tag="rr")
                            nc.vector.reciprocal(rr, rsum)
                            wN = tr.tile([128, T], F32, name="wN", tag="wN")
                            nc.vector.tensor_scalar_mul(
                                wN[:, 0:w_], wU[:, 0:w_], rr
                            )
                            for sc in range(2 if tc_ else 1):
                                pt = ps.tile([128, 128], F32, name="pa", tag="pa")
                                nc.tensor.transpose(
                                    pt, wN[:, sc * 128 : (sc + 1) * 128], id_sb
                                )
                                nc.scalar.copy(
                                    wTs[par][sc][:, tc_ * 128 : (tc_ + 1) * 128],
                                    pt,
                                )
                    for par in range(2):
                        hh = 2 * ch + par
                        o_ = par * 64
                        pA = ps.tile([64, T], F32, name="pa", tag="pa")
                        nc.tensor.matmul(
                            pA, vt[ib * 2][:, hh * 64 : (hh + 1) * 64],
                            wTs[par][0], start=True, stop=False,
                        )
                        nc.tensor.matmul(
                            pA[:, 128:T],
                            vt[ib * 2 + 1][:, hh * 64 : (hh + 1) * 64],
                            wTs[par][1][:, 128:T], start=False, stop=True,
                        )
                        nc.scalar.copy(
                            acT[ch][o_ : o_ + 64, ib * T : (ib + 1) * T], pA
                        )

            # ---- stage 4: proj + residual -> y ----
            yt = [
                sb.tile([128, C], F32, name=f"y{i}", tag=f"y{i}")
                for i in range(4)
            ]
            for i in range(4):
                pP = ps.tile([128, C], F32, name="pa", tag="pa")
                for k in range(KC):
                    nc.tensor.matmul(
                        pP, acT[k][:, i * 128 : (i + 1) * 128], wp_sb[k],
                        start=(k == 0), stop=(k == KC - 1),
                    )
                nc.vector.tensor_add(yt[i], pP, xt[i])
                if bpb_sb is not None:
                    nc.vector.tensor_add(yt[i], yt[i], bpb_sb)

            # ---- stage 5: LN2 + transpose -> h2T ----
            h2T = [
                sb.tile([128, 2 * T], F32R, name=f"h2T{c}", tag=f"h2T{c}", bufs=2)
                for c in range(KC)
            ]
            h2_ = [
                sb.tile([128, C], F32, name=f"h2{i}", tag=f"h2{i}")
                for i in range(4)
            ]
            layernorm4(h2_, yt, g2_sb, b2ln_sb)
            for i in range(4):
                transpose_into(h2T, h2_[i], i)

            # ---- stage 6: MLP ----
            psY = [
                psy.tile([128, C], F32, name=f"psY{i}", tag=f"psY{i}")
                for i in range(4)
            ]
            for m in range(KH):
                pM = ps.tile([128, 2 * T], F32, name="pa", tag="pa")
                for k in range(KC):
                    nc.tensor.matmul(
                        pM, w1_sb[k][:, m * 128 : (m + 1) * 128], h2T[k],
                        start=(k == 0), stop=(k == KC - 1),
                    )
                m1r = tr.tile([128, 2 * T], F32R, name="m1r", tag="m1r")
                nc.vector.tensor_scalar(
                    out=m1r, in0=pM,
                    scalar1=(b1c_sb[:, m : m + 1] if use_b1 else 0.0),
                    scalar2=0.0, op0=ALU.add, op1=ALU.max,
                )
                for i in range(4):
                    nc.tensor.matmul(
                        psY[i], m1r[:, i * 128 : (i + 1) * 128], w2_sb[m],
                        start=(m == 0), stop=(m == KH - 1),
                    )

            # ---- stage 7: residual + store ----
            for i in range(4):
                ot = sb.tile([128, C], F32, name=f"ot{i}", tag=f"ot{i}", bufs=2)
                nc.vector.tensor_add(ot, psY[i], yt[i])
                if b2b_sb is not None:
                    nc.vector.tensor_add(ot, ot, b2b_sb)
                nc.sync.dma_start(
                    out=out[bs[i], tch[i] * 128 : (tch[i] + 1) * 128, :],
                    in_=ot,
                )
    nc.compile()
    return nc


def _host_prep(inputs):
    f = np.float32
    x = np.ascontiguousarray(inputs["x"], dtype=f)
    wq_full = np.ascontiguousarray(
        np.asarray(inputs["wq"], dtype=f).transpose(1, 0, 2).reshape(C, C)
        * (C ** -0.5)
    )
    wk_full = np.ascontiguousarray(
        np.asarray(inputs["wk"], dtype=f).transpose(1, 0, 2).reshape(C, C)
    )
    wv_full = np.ascontiguousarray(
        np.asarray(inputs["wv"], dtype=f).transpose(1, 0, 2).reshape(C, C)
    )
    wp = np.ascontiguousarray(inputs["w_proj"], dtype=f)
    w1 = np.ascontiguousarray(inputs["w1"], dtype=f)
    w2 = np.ascontiguousarray(inputs["w2"], dtype=f)
    tile128 = lambda v: np.ascontiguousarray(
        np.broadcast_to(np.asarray(v, dtype=f), (128, C))
    )
    g1 = tile128(inputs["ln1_g"])
    b1ln = tile128(inputs["ln1_b"])
    g2 = tile128(inputs["ln2_g"])
    b2ln = tile128(inputs["ln2_b"])
    bpb = tile128(inputs["b_proj"])
    b2b = tile128(inputs["b2"])
    b1c = np.ascontiguousarray(np.asarray(inputs["b1"], dtype=f).reshape(KH, 128).T)
    tril = np.tril(np.ones((128, 128), dtype=f))
    m0 = np.concatenate([tril, np.zeros((128, 128), dtype=f)], axis=1)
    m1m = np.concatenate([np.ones((128, 128), dtype=f), tril], axis=1)
    ident = np.eye(128, dtype=f)

    flags = (
        bool(not np.all(np.asarray(inputs["ln1_g"]) == 1.0)),
        bool(np.any(np.asarray(inputs["ln1_b"]))),
        bool(not np.all(np.asarray(inputs["ln2_g"]) == 1.0)),
        bool(np.any(np.asarray(inputs["ln2_b"]))),
        bool(np.any(np.asarray(inputs["b_proj"]))),
        bool(np.any(np.asarray(inputs["b1"]))),
        bool(np.any(np.asarray(inputs["b2"]))),
    )
    shared = dict(
        wq=wq_full, wk=wk_full, wv=wv_full, wp=wp, w1=w1, w2=w2,
        g1=g1, b1ln=b1ln, g2=g2, b2ln=b2ln, bpb=bpb, b2b=b2b, b1c=b1c,
        m0=m0, m1m=m1m, ident=ident,
    )
    in_maps = []
    for i in range(NCORES):
        m = dict(shared)
        m["x"] = np.ascontiguousarray(x[i * BL : (i + 1) * BL])
        in_maps.append(m)
    return in_maps, flags


_NC_CACHE = {}


def _get_program(flags):
    if flags not in _NC_CACHE:
        _NC_CACHE[flags] = build_program(*flags)
    return _NC_CACHE[flags]


def run(inputs, **spmd_kwargs):
    from concourse.bass_utils import run_bass_kernel_spmd

    in_maps, flags = _host_prep(inputs)
    nc = _get_program(flags)
    bkr = run_bass_kernel_spmd(nc, in_maps, list(range(NCORES)), **spmd_kwargs)
    outs = [bkr.results[i]["out"] for i in range(NCORES)]
    return np.concatenate(outs, axis=0).astype(np.float32), bkr


def kernel(**inputs):
    full, _ = run(inputs)
    return full



# revision 4
# speedup vs baseline: 1.0765x; 1.0765x over previous
"""Trainium2 Bass kernel for a dense transformer block (B=128, T=256, C=384, H=6).

Sharding: data-parallel over batch across 8 NeuronCores (16 batches/core),
identical SPMD program per core, no collectives.

v2 design (vs v1 baseline):
  - Attention computed in transposed orientation: scores are produced as
    S^T[key, query] directly (lhsT=k, rhs=q), so the softmax weights feed the
    attention matmul (lhsT=v, rhs=expS^T) with NO per-tile PE transposes and
    NO scalar-engine copies of the weights.
  - Softmax: exp + causal mask fused into ONE DVE op per score tile:
    out_i32 = rint(scores * EXP_S + maskbias), where maskbias holds the
    Schraudolph exponent bias (127*2^23) for allowed entries and a tiny
    exponent (40*2^23 -> ~1e-27) for masked entries. Bitcast to float gives
    exp(score) (plain Schraudolph, ~2-3% max rel err; validated end-to-end
    rel err ~8e-3 vs 2e-2 budget).
  - Softmax denominators via ones-column matmuls on the PE (sum over keys =
    partition-dim reduction). One reciprocal per (batch, head-pair) group;
    1/rowsum broadcast across partitions by a tiny K=1 matmul; the normalize
    multiply doubles as the PSUM->SBUF move of the attention output.
  - All big matmuls in bf16 (weights converted host-side; activations
    converted for free in the PSUM->SBUF copies). fp32 residual spine.
    exp output stays f32r (bit-trick requires f32 layout).
  - LayerNorm applies on GpSimd (SBUF->SBUF), stats on DVE, copies on ACT:
    spreads elementwise work across three engines.
  - Head pairs packed: score matmuls row-tiled (K=64 x2 concurrent),
    attention/broadcast matmuls col-tiled to stack the two heads of a chunk
    into partitions 0:64 / 64:128 of one PSUM bank.
"""

import numpy as np

import concourse.bass as bass
import concourse.mybir as mybir
from concourse import bacc
from concourse.tile import TileContext
from contextlib import ExitStack

B, T, C = 128, 256, 384
H, D = 6, 64
FF = 4 * C
NCORES = 8
BL = B // NCORES  # 16
NPAIR = BL // 2  # 8
KC = C // 128  # 3
KH = FF // 128  # 12
EPS = 1e-5
F32 = mybir.dt.float32
F32R = mybir.dt.float32r
BF16 = mybir.dt.bfloat16
I32 = mybir.dt.int32
I16 = mybir.dt.int16
ALU = mybir.AluOpType

# int16 Schraudolph: bf16 shares f32's 8-bit exponent; exp(s) ~ bitcast
# int16(rint(s * 2^7/ln2 + 127*2^7)) as bf16. Masked entries get a tiny
# exponent bias (40*2^7 -> ~1e-26).
EXP_S = float(2**7 / np.log(2.0))
EXP_ALLOW = float(127 * 2**7)
EXP_MASKED = float(40 * 2**7)
SQRT_MAGIC = 0x1FBD1DF5

# col-tiling (tile_position=(0,64)) stacks the two heads of a chunk into one
# PSUM bank; if unsupported, set False to fall back to per-head tiles.
COLTILE = True


def build_program(use_g1, use_b1ln, use_g2, use_b2ln, use_bp, use_b1, use_b2):
    nc = bacc.Bacc(None)
    x = nc.declare_dram_parameter("x", [BL, T, C], F32, isOutput=False)
    wq = nc.declare_dram_parameter("wq", [C, C], BF16, isOutput=False)
    wk = nc.declare_dram_parameter("wk", [C, C], BF16, isOutput=False)
    wv = nc.declare_dram_parameter("wv", [C, C], BF16, isOutput=False)
    wp = nc.declare_dram_parameter("wp", [C, C], BF16, isOutput=False)
    w1 = nc.declare_dram_parameter("w1", [C, FF], BF16, isOutput=False)
    w2 = nc.declare_dram_parameter("w2", [FF, C], BF16, isOutput=False)
    g1 = nc.declare_dram_parameter("g1", [128, C], F32, isOutput=False)
    b1ln = nc.declare_dram_parameter("b1ln", [128, C], F32, isOutput=False)
    g2 = nc.declare_dram_parameter("g2", [128, C], F32, isOutput=False)
    b2ln = nc.declare_dram_parameter("b2ln", [128, C], F32, isOutput=False)
    bpb = nc.declare_dram_parameter("bpb", [128, C], F32, isOutput=False)
    b2b = nc.declare_dram_parameter("b2b", [128, C], F32, isOutput=False)
    b1c = nc.declare_dram_parameter("b1c", [128, KH], F32, isOutput=False)
    mb = nc.declare_dram_parameter("mb", [128, 2 * T], F32, isOutput=False)
    identb = nc.declare_dram_parameter("identb", [128, 128], BF16, isOutput=False)
    ones1r = nc.declare_dram_parameter("ones1r", [1, 128], BF16, isOutput=False)
    onescol = nc.declare_dram_parameter("onescol", [128, 1], BF16, isOutput=False)
    out = nc.declare_dram_parameter("out", [BL, T, C], F32, isOutput=True)

    with TileContext(nc) as tc, ExitStack() as ctx:
        wts = ctx.enter_context(tc.tile_pool(name="wts", bufs=1))
        sb = ctx.enter_context(tc.tile_pool(name="sb", bufs=1))
        st = ctx.enter_context(tc.tile_pool(name="st", bufs=4))
        ps = ctx.enter_context(tc.tile_pool(name="ps", bufs=8, space="PSUM"))

        def load_chunks(dram, n, width, tagp, dt):
            tiles = []
            for k in range(n):
                t_ = wts.tile([128, width], dt, name=f"{tagp}{k}", tag=f"{tagp}{k}")
                nc.sync.dma_start(out=t_, in_=dram[k * 128 : (k + 1) * 128, :])
                tiles.append(t_)
            return tiles

        wq_sb = load_chunks(wq, KC, C, "wq", BF16)
        wk_sb = load_chunks(wk, KC, C, "wk", BF16)
        wv_sb = load_chunks(wv, KC, C, "wv", BF16)
        wp_sb = load_chunks(wp, KC, C, "wp", BF16)
        w1_sb = load_chunks(w1, KC, FF, "w1", BF16)
        w2_sb = load_chunks(w2, KH, C, "w2", BF16)

        def load_one(dram, shape, tag, dt=F32):
            t_ = wts.tile(shape, dt, name=tag, tag=tag)
            nc.sync.dma_start(out=t_, in_=dram[:, :])
            return t_

        g1_sb = load_one(g1, [128, C], "g1") if use_g1 else None
        b1ln_sb = load_one(b1ln, [128, C], "b1ln") if use_b1ln else None
        g2_sb = load_one(g2, [128, C], "g2") if use_g2 else None
        b2ln_sb = load_one(b2ln, [128, C], "b2ln") if use_b2ln else None
        bpb_sb = load_one(bpb, [128, C], "bpb") if use_bp else None
        b2b_sb = load_one(b2b, [128, C], "b2b") if use_b2 else None
        b1c_sb = load_one(b1c, [128, KH], "b1c") if use_b1 else None
        mb_sb = load_one(mb, [128, 2 * T], "mb")
        id_sb = load_one(identb, [128, 128], "identb", BF16)
        ones1r_sb = load_one(ones1r, [1, 128], "ones1r", BF16)
        onescol_sb = load_one(onescol, [128, 1], "onescol", BF16)

        for p in range(NPAIR):
            bs = [2 * p, 2 * p, 2 * p + 1, 2 * p + 1]
            tch = [0, 1, 0, 1]

            def batched_rstd(mv8):
                """[128,8] interleaved (mean,var) x4 -> rstd4 [128,4]."""
                mv_v = mv8.rearrange("p (i two) -> p i two", two=2)
                var4 = mv_v[:, :, 1]
                vpe = st.tile([128, 4], F32, name="vpe", tag="vpe")
                nc.vector.tensor_scalar(
                    out=vpe, in0=var4, scalar1=EPS, scalar2=None, op0=ALU.add
                )
                s0h = st.tile([128, 4], I32, name="s0h", tag="s0h")
                nc.vector.tensor_scalar(
                    out=s0h, in0=vpe.bitcast(I32), scalar1=1, scalar2=None,
                    op0=ALU.logical_shift_right,
                )
                s0i = st.tile([128, 4], I32, name="s0i", tag="s0i")
                nc.vector.tensor_scalar(
                    out=s0i, in0=s0h, scalar1=SQRT_MAGIC, scalar2=None,
                    op0=ALU.add,
                )
                cur = s0i.bitcast(F32)
                for hi in range(2):
                    r_ = st.tile([128, 4], F32, name=f"hr{hi}", tag=f"hr{hi}")
                    nc.vector.reciprocal(r_, cur)
                    t_ = st.tile([128, 4], F32, name=f"ht{hi}", tag=f"ht{hi}")
                    nc.vector.tensor_mul(t_, vpe, r_)
                    s_ = st.tile([128, 4], F32, name=f"hs{hi}", tag=f"hs{hi}")
                    nc.vector.tensor_add(s_, t_, cur)
                    sh = st.tile([128, 4], F32, name=f"hh{hi}", tag=f"hh{hi}")
                    nc.vector.tensor_scalar_mul(sh, s_, 0.5)
                    cur = sh
                rstd4 = st.tile([128, 4], F32, name="rstd4", tag="rstd4")
                nc.vector.reciprocal(rstd4, cur)
                return rstd4

            def layernorm4(dsts, srcs, g_sb, b_sb):
                mv8 = st.tile([128, 8], F32, name="mv8", tag="mv8")
                for i in range(4):
                    stats = st.tile([128, 6], F32, name="lst", tag="lst")
                    nc.vector.bn_stats(stats, srcs[i])
                    nc.vector.bn_aggr(mv8[:, 2 * i : 2 * i + 2], stats)
                rstd4 = batched_rstd(mv8)
                for i in range(4):
                    nc.gpsimd.tensor_scalar(
                        out=dsts[i], in0=srcs[i],
                        scalar1=mv8[:, 2 * i : 2 * i + 1],
                        scalar2=rstd4[:, i : i + 1],
                        op0=ALU.subtract, op1=ALU.mult,
                    )
                    if g_sb is not None:
                        nc.gpsimd.tensor_mul(dsts[i], dsts[i], g_sb)
                    if b_sb is not None:
                        nc.gpsimd.tensor_add(dsts[i], dsts[i], b_sb)

            def transpose_into(dstT, src, i):
                for c in range(KC):
                    pt = ps.tile([128, 128], BF16, name="pa", tag="pa")
                    nc.tensor.transpose(
                        pt, src[:, c * 128 : (c + 1) * 128], id_sb
                    )
                    nc.scalar.copy(dstT[c][:, i * 128 : (i + 1) * 128], pt)

            # ---- stage 1: load x, LN1, transpose -> hT (bf16, C-major) ----
            xt = [
                sb.tile([128, C], F32, name=f"xt{i}", tag=f"xt{i}", bufs=2)
                for i in range(4)
            ]
            for i in range(4):
                nc.sync.dma_start(
                    out=xt[i],
                    in_=x[bs[i], tch[i] * 128 : (tch[i] + 1) * 128, :],
                )
            hT = [
                sb.tile([128, 2 * T], BF16, name=f"hT{c}", tag=f"hT{c}", bufs=2)
                for c in range(KC)
            ]
            ht_ = [
                sb.tile([128, C], BF16, name=f"h{i}", tag=f"h{i}", bufs=2)
                for i in range(4)
            ]
            layernorm4(ht_, xt, g1_sb, b1ln_sb)
            for i in range(4):
                transpose_into(hT, ht_[i], i)

            # ---- stage 2: q^T, k^T (C-major bf16), v (token-major f32r) ----
            qT = [
                sb.tile([128, 2 * T], BF16, name=f"qT{m}", tag=f"qT{m}", bufs=2)
                for m in range(KC)
            ]
            kT = [
                sb.tile([128, 2 * T], BF16, name=f"kT{m}", tag=f"kT{m}", bufs=2)
                for m in range(KC)
            ]
            for m in range(KC):
                pq = ps.tile([128, 2 * T], F32, name="pa", tag="pa")
                for k in range(KC):
                    nc.tensor.matmul(
                        pq, wq_sb[k][:, m * 128 : (m + 1) * 128], hT[k],
                        start=(k == 0), stop=(k == KC - 1),
                    )
                nc.scalar.copy(qT[m], pq)
                pk = ps.tile([128, 2 * T], F32, name="pa", tag="pa")
                for k in range(KC):
                    nc.tensor.matmul(
                        pk, wk_sb[k][:, m * 128 : (m + 1) * 128], hT[k],
                        start=(k == 0), stop=(k == KC - 1),
                    )
                nc.scalar.copy(kT[m], pk)
            vt = [
                sb.tile([128, C], BF16, name=f"v{i}", tag=f"v{i}", bufs=2)
                for i in range(4)
            ]
            for i in range(4):
                pv = ps.tile([128, C], F32, name="pa", tag="pa")
                for k in range(KC):
                    nc.tensor.matmul(
                        pv, hT[k][:, i * 128 : (i + 1) * 128], wv_sb[k],
                        start=(k == 0), stop=(k == KC - 1),
                    )
                nc.scalar.copy(vt[i], pv)

            # ---- stage 3: attention, S^T orientation ----
            acT = [
                sb.tile([128, 2 * T], BF16, name=f"acT{c}", tag=f"acT{c}", bufs=2)
                for c in range(KC)
            ]
            for ib in range(2):
                q0 = ib * T
                for ch in range(KC):
                    eT = []
                    for par in range(2):
                        off = par * 64
                        # scores: S^T[key, query]; kc0 block cols 0:256
                        # (keys 0:128 x all q), kc1 block cols 256:512
                        # (keys 128:256 x all q, q<128 masked)
                        pS = ps.tile(
                            [128, 2 * T], F32, name="pa", tag="pa"
                        )
                        for kc in range(2):
                            nc.tensor.matmul(
                                pS[:, kc * T : (kc + 1) * T],
                                kT[ch][
                                    off : off + 64,
                                    q0 + kc * 128 : q0 + (kc + 1) * 128,
                                ],
                                qT[ch][off : off + 64, q0 : q0 + T],
                                start=True, stop=True,
                                tile_position=(off, 0),
                            )
                        # exp + causal mask fused: rint(s*EXP_S + maskbias)
                        e_ = sb.tile(
                            [128, 2 * T], I16,
                            name=f"eT{par}", tag=f"eT{par}", bufs=2,
                        )
                        nc.vector.scalar_tensor_tensor(
                            out=e_, in0=pS, scalar=EXP_S, in1=mb_sb,
                            op0=ALU.mult, op1=ALU.add,
                        )
                        eT.append(e_.bitcast(BF16))

                    # rowsums: ones-column matmuls, both heads -> one bank
                    rs = ps.tile([1, 2 * T], F32, name="pa", tag="pa")
                    for par in range(2):
                        for kc in range(2):
                            nc.tensor.matmul(
                                rs[0:1, par * T : (par + 1) * T],
                                onescol_sb,
                                eT[par][:, kc * T : (kc + 1) * T],
                                start=(kc == 0), stop=(kc == 1),
                            )
                    rr = sb.tile([1, 2 * T], BF16, name="rr", tag="rr", bufs=2)
                    with nc.allow_low_precision("bf16 1/rowsum, ~0.4% ok"):
                        nc.vector.reciprocal(rr, rs)

                    if COLTILE:
                        # attn out + 1/rowsum broadcast, heads stacked in one
                        # bank via col-tiling
                        pA = ps.tile([128, T], F32, name="pa", tag="pa")
                        rrB = ps.tile([128, T], F32, name="pa", tag="pa")
                        for par in range(2):
                            off = par * 64
                            hh = 2 * ch + par
                            for kc in range(2):
                                nc.tensor.matmul(
                                    pA[off : off + 64, :],
                                    vt[ib * 2 + kc][:, hh * 64 : hh * 64 + 64],
                                    eT[par][:, kc * T : (kc + 1) * T],
                                    start=(kc == 0), stop=(kc == 1),
                                    tile_position=(0, off),
                                )
                            nc.tensor.matmul(
                                rrB[off : off + 64, :],
                                ones1r_sb[0:1, 0:64],
                                rr[0:1, par * T : (par + 1) * T],
                                start=True, stop=True,
                                tile_position=(0, off),
                            )
                        # DVE may read only ONE operand from PSUM: stage the
                        # broadcast through SBUF via an ACT copy
                        rrS = sb.tile([128, T], BF16, name="rrS", tag="rrS",
                                      bufs=3)
                        nc.scalar.copy(rrS, rrB)
                        nc.vector.tensor_mul(
                            acT[ch][:, q0 : q0 + T], pA, rrS
                        )
                    else:
                        # fallback: full-height matmuls (garbage in the other
                        # head's rows, never read), no col-tiling needed
                        for par in range(2):
                            o_ = par * 64
                            pA = ps.tile([128, T], F32, name="pa", tag="pa")
                            for kc in range(2):
                                nc.tensor.matmul(
                                    pA,
                                    vt[ib * 2 + kc][:, ch * 128 : (ch + 1) * 128],
                                    eT[par][:, kc * T : (kc + 1) * T],
                                    start=(kc == 0), stop=(kc == 1),
                                )
                            rrB = ps.tile([128, T], F32, name="pa", tag="pa")
                            nc.tensor.matmul(
                                rrB, ones1r_sb,
                                rr[0:1, par * T : (par + 1) * T],
                                start=True, stop=True,
                            )
                            rrS = sb.tile([128, T], BF16, name="rrS",
                                          tag="rrS", bufs=3)
                            nc.scalar.copy(rrS, rrB)
                            nc.vector.tensor_mul(
                                acT[ch][o_ : o_ + 64, q0 : q0 + T],
                                pA[o_ : o_ + 64, :],
                                rrS[o_ : o_ + 64, :],
                            )

            # ---- stage 4: proj + residual -> y ----
            yt = [
                sb.tile([128, C], F32, name=f"y{i}", tag=f"y{i}", bufs=2)
                for i in range(4)
            ]
            for i in range(4):
                pP = ps.tile([128, C], F32, name="pa", tag="pa")
                for k in range(KC):
                    nc.tensor.matmul(
                        pP, acT[k][:, i * 128 : (i + 1) * 128], wp_sb[k],
                        start=(k == 0), stop=(k == KC - 1),
                    )
                nc.vector.tensor_add(yt[i], pP, xt[i])
                if bpb_sb is not None:
                    nc.vector.tensor_add(yt[i], yt[i], bpb_sb)

            # ---- stage 5: LN2 + transpose -> h2T ----
            h2T = [
                sb.tile([128, 2 * T], BF16, name=f"h2T{c}", tag=f"h2T{c}", bufs=2)
                for c in range(KC)
            ]
            h2_ = [
                sb.tile([128, C], BF16, name=f"h2{i}", tag=f"h2{i}", bufs=2)
                for i in range(4)
            ]
            layernorm4(h2_, yt, g2_sb, b2ln_sb)
            for i in range(4):
                transpose_into(h2T, h2_[i], i)

            # ---- stage 6: MLP ----
            m1r = [
                sb.tile([128, 2 * T], BF16, name=f"m1r{m}", tag=f"m1r{m}", bufs=2)
                for m in range(KH)
            ]
            for m in range(KH):
                pM = ps.tile([128, 2 * T], F32, name="pa", tag="pa")
                for k in range(KC):
                    nc.tensor.matmul(
                        pM, w1_sb[k][:, m * 128 : (m + 1) * 128], h2T[k],
                        start=(k == 0), stop=(k == KC - 1),
                    )
                nc.vector.tensor_scalar(
                    out=m1r[m], in0=pM,
                    scalar1=(b1c_sb[:, m : m + 1] if use_b1 else 0.0),
                    scalar2=0.0, op0=ALU.add, op1=ALU.max,
                )

            # ---- stage 6b/7: MLP2 (one PSUM bank at a time) + residual ----
            for i in range(4):
                psY = ps.tile([128, C], F32, name="pa", tag="pa")
                for m in range(KH):
                    nc.tensor.matmul(
                        psY, m1r[m][:, i * 128 : (i + 1) * 128], w2_sb[m],
                        start=(m == 0), stop=(m == KH - 1),
                    )
                ot = sb.tile([128, C], F32, name=f"ot{i}", tag=f"ot{i}", bufs=2)
                nc.vector.tensor_add(ot, psY, yt[i])
                if b2b_sb is not None:
                    nc.vector.tensor_add(ot, ot, b2b_sb)
                nc.sync.dma_start(
                    out=out[bs[i], tch[i] * 128 : (tch[i] + 1) * 128, :],
                    in_=ot,
                )
    nc.compile()
    return nc


def _to_bf16(a):
    import ml_dtypes

    return np.ascontiguousarray(a.astype(ml_dtypes.bfloat16))


def _host_prep(inputs):
    f = np.float32
    x = np.ascontiguousarray(inputs["x"], dtype=f)
    wq_full = (
        np.asarray(inputs["wq"], dtype=f).transpose(1, 0, 2).reshape(C, C)
        * (C ** -0.5)
    )
    wk_full = np.asarray(inputs["wk"], dtype=f).transpose(1, 0, 2).reshape(C, C)
    wv_full = np.asarray(inputs["wv"], dtype=f).transpose(1, 0, 2).reshape(C, C)
    wp_ = np.asarray(inputs["w_proj"], dtype=f)
    w1_ = np.asarray(inputs["w1"], dtype=f)
    w2_ = np.asarray(inputs["w2"], dtype=f)
    tile128 = lambda v: np.ascontiguousarray(
        np.broadcast_to(np.asarray(v, dtype=f), (128, C))
    )
    g1 = tile128(inputs["ln1_g"])
    b1ln = tile128(inputs["ln1_b"])
    g2 = tile128(inputs["ln2_g"])
    b2ln = tile128(inputs["ln2_b"])
    bpb = tile128(inputs["b_proj"])
    b2b = tile128(inputs["b2"])
    b1c = np.ascontiguousarray(np.asarray(inputs["b1"], dtype=f).reshape(KH, 128).T)

    # exp maskbias: [128, 512]; cols 0:256 = keys 0:128 (allow q >= s),
    # cols 256:512 = keys 128:256 (allow q >= 128 + s)
    s_idx = np.arange(128)[:, None]
    q_idx = np.arange(T)[None, :]
    mb0 = np.where(q_idx >= s_idx, EXP_ALLOW, EXP_MASKED)
    mb1 = np.where(q_idx >= 128 + s_idx, EXP_ALLOW, EXP_MASKED)
    mbias = np.ascontiguousarray(
        np.concatenate([mb0, mb1], axis=1).astype(f)
    )
    import ml_dtypes

    identb = np.ascontiguousarray(np.eye(128, dtype=ml_dtypes.bfloat16))
    ones1r = np.ones((1, 128), dtype=ml_dtypes.bfloat16)
    onescol = np.ones((128, 1), dtype=ml_dtypes.bfloat16)

    flags = (
        bool(not np.all(np.asarray(inputs["ln1_g"]) == 1.0)),
        bool(np.any(np.asarray(inputs["ln1_b"]))),
        bool(not np.all(np.asarray(inputs["ln2_g"]) == 1.0)),
        bool(np.any(np.asarray(inputs["ln2_b"]))),
        bool(np.any(np.asarray(inputs["b_proj"]))),
        bool(np.any(np.asarray(inputs["b1"]))),
        bool(np.any(np.asarray(inputs["b2"]))),
    )
    shared = dict(
        wq=_to_bf16(wq_full), wk=_to_bf16(wk_full), wv=_to_bf16(wv_full),
        wp=_to_bf16(wp_), w1=_to_bf16(w1_), w2=_to_bf16(w2_),
        g1=g1, b1ln=b1ln, g2=g2, b2ln=b2ln, bpb=bpb, b2b=b2b, b1c=b1c,
        mb=mbias, identb=identb, ones1r=ones1r, onescol=onescol,
    )
    in_maps = []
    for i in range(NCORES):
        m = dict(shared)
        m["x"] = np.ascontiguousarray(x[i * BL : (i + 1) * BL])
        in_maps.append(m)
    return in_maps, flags


_NC_CACHE = {}


def _get_program(flags):
    if flags not in _NC_CACHE:
        _NC_CACHE[flags] = build_program(*flags)
    return _NC_CACHE[flags]


def run(inputs, **spmd_kwargs):
    from concourse.bass_utils import run_bass_kernel_spmd

    in_maps, flags = _host_prep(inputs)
    nc = _get_program(flags)
    bkr = run_bass_kernel_spmd(nc, in_maps, list(range(NCORES)), **spmd_kwargs)
    outs = [bkr.results[i]["out"] for i in range(NCORES)]
    return np.concatenate(outs, axis=0).astype(np.float32), bkr


def kernel(**inputs):
    full, _ = run(inputs)
    return full


# revision 5
# speedup vs baseline: 1.8456x; 1.7145x over previous
"""Trainium2 Bass kernel for a dense transformer block (B=128, T=256, C=384, H=6).

Sharding: data-parallel over batch across 8 NeuronCores (16 batches/core),
identical SPMD program per core, no collectives.

v2 design (vs v1 baseline):
  - Attention computed in transposed orientation: scores are produced as
    S^T[key, query] directly (lhsT=k, rhs=q), so the softmax weights feed the
    attention matmul (lhsT=v, rhs=expS^T) with NO per-tile PE transposes and
    NO scalar-engine copies of the weights.
  - Softmax: exp + causal mask fused into ONE DVE op per score tile:
    out_i32 = rint(scores * EXP_S + maskbias), where maskbias holds the
    Schraudolph exponent bias (127*2^23) for allowed entries and a tiny
    exponent (40*2^23 -> ~1e-27) for masked entries. Bitcast to float gives
    exp(score) (plain Schraudolph, ~2-3% max rel err; validated end-to-end
    rel err ~8e-3 vs 2e-2 budget).
  - Softmax denominators via ones-column matmuls on the PE (sum over keys =
    partition-dim reduction). One reciprocal per (batch, head-pair) group;
    1/rowsum broadcast across partitions by a tiny K=1 matmul; the normalize
    multiply doubles as the PSUM->SBUF move of the attention output.
  - All big matmuls in bf16 (weights converted host-side; activations
    converted for free in the PSUM->SBUF copies). fp32 residual spine.
    exp output stays f32r (bit-trick requires f32 layout).
  - LayerNorm applies on GpSimd (SBUF->SBUF), stats on DVE, copies on ACT:
    spreads elementwise work across three engines.
  - Head pairs packed: score matmuls row-tiled (K=64 x2 concurrent),
    attention/broadcast matmuls col-tiled to stack the two heads of a chunk
    into partitions 0:64 / 64:128 of one PSUM bank.
"""

import numpy as np

import concourse.bass as bass
import concourse.mybir as mybir
from concourse import bacc
from concourse.tile import TileContext
from contextlib import ExitStack

B, T, C = 128, 256, 384
H, D = 6, 64
FF = 4 * C
NCORES = 8
BL = B // NCORES  # 16
NPAIR = BL // 2  # 8
KC = C // 128  # 3
KH = FF // 128  # 12
EPS = 1e-5
F32 = mybir.dt.float32
F32R = mybir.dt.float32r
BF16 = mybir.dt.bfloat16
I32 = mybir.dt.int32
I16 = mybir.dt.int16
ALU = mybir.AluOpType

# int16 Schraudolph: bf16 shares f32's 8-bit exponent; exp(s) ~ bitcast
# int16(rint(s * 2^7/ln2 + 127*2^7)) as bf16. Masked entries get a tiny
# exponent bias (40*2^7 -> ~1e-26).
EXP_S = float(2**7 / np.log(2.0))
EXP_ALLOW = float(127 * 2**7)
EXP_MASKED = float(40 * 2**7)
SQRT_MAGIC = 0x1FBD1DF5

# col-tiling (tile_position=(0,64)) stacks the two heads of a chunk into one
# PSUM bank; if unsupported, set False to fall back to per-head tiles.
COLTILE = True


def build_program(use_g1, use_b1ln, use_g2, use_b2ln, use_bp, use_b1, use_b2):
    nc = bacc.Bacc(None)
    x = nc.declare_dram_parameter("x", [BL, T, C], F32, isOutput=False)
    wq = nc.declare_dram_parameter("wq", [C, C], BF16, isOutput=False)
    wk = nc.declare_dram_parameter("wk", [C, C], BF16, isOutput=False)
    wv = nc.declare_dram_parameter("wv", [C, C], BF16, isOutput=False)
    wp = nc.declare_dram_parameter("wp", [C, C], BF16, isOutput=False)
    w1 = nc.declare_dram_parameter("w1", [C, FF], BF16, isOutput=False)
    w2 = nc.declare_dram_parameter("w2", [FF, C], BF16, isOutput=False)
    g1 = nc.declare_dram_parameter("g1", [128, C], F32, isOutput=False)
    b1ln = nc.declare_dram_parameter("b1ln", [128, C], F32, isOutput=False)
    g2 = nc.declare_dram_parameter("g2", [128, C], F32, isOutput=False)
    b2ln = nc.declare_dram_parameter("b2ln", [128, C], F32, isOutput=False)
    bpb = nc.declare_dram_parameter("bpb", [128, C], F32, isOutput=False)
    b2b = nc.declare_dram_parameter("b2b", [128, C], F32, isOutput=False)
    b1c = nc.declare_dram_parameter("b1c", [128, KH], F32, isOutput=False)
    mb = nc.declare_dram_parameter("mb", [128, 2 * T], F32, isOutput=False)
    identb = nc.declare_dram_parameter("identb", [128, 128], BF16, isOutput=False)
    ones1r = nc.declare_dram_parameter("ones1r", [1, 128], BF16, isOutput=False)
    onescol = nc.declare_dram_parameter("onescol", [128, 1], BF16, isOutput=False)
    out = nc.declare_dram_parameter("out", [BL, T, C], F32, isOutput=True)

    with TileContext(nc) as tc, ExitStack() as ctx:
        wts = ctx.enter_context(tc.tile_pool(name="wts", bufs=1))
        sb = ctx.enter_context(tc.tile_pool(name="sb", bufs=1))
        st = ctx.enter_context(tc.tile_pool(name="st", bufs=4))
        ps = ctx.enter_context(tc.tile_pool(name="ps", bufs=8, space="PSUM"))

        def load_chunks(dram, n, width, tagp, dt):
            tiles = []
            for k in range(n):
                t_ = wts.tile([128, width], dt, name=f"{tagp}{k}", tag=f"{tagp}{k}")
                nc.sync.dma_start(out=t_, in_=dram[k * 128 : (k + 1) * 128, :])
                tiles.append(t_)
            return tiles

        wq_sb = load_chunks(wq, KC, C, "wq", BF16)
        wk_sb = load_chunks(wk, KC, C, "wk", BF16)
        wv_sb = load_chunks(wv, KC, C, "wv", BF16)
        wp_sb = load_chunks(wp, KC, C, "wp", BF16)
        w1_sb = load_chunks(w1, KC, FF, "w1", BF16)
        w2_sb = load_chunks(w2, KH, C, "w2", BF16)

        def load_one(dram, shape, tag, dt=F32):
            t_ = wts.tile(shape, dt, name=tag, tag=tag)
            nc.sync.dma_start(out=t_, in_=dram[:, :])
            return t_

        g1_sb = load_one(g1, [128, C], "g1") if use_g1 else None
        b1ln_sb = load_one(b1ln, [128, C], "b1ln") if use_b1ln else None
        g2_sb = load_one(g2, [128, C], "g2") if use_g2 else None
        b2ln_sb = load_one(b2ln, [128, C], "b2ln") if use_b2ln else None
        bpb_sb = load_one(bpb, [128, C], "bpb") if use_bp else None
        b2b_sb = load_one(b2b, [128, C], "b2b") if use_b2 else None
        b1c_sb = load_one(b1c, [128, KH], "b1c") if use_b1 else None
        mb_sb = load_one(mb, [128, 2 * T], "mb")
        id_sb = load_one(identb, [128, 128], "identb", BF16)
        ones1r_sb = load_one(ones1r, [1, 128], "ones1r", BF16)
        onescol_sb = load_one(onescol, [128, 1], "onescol", BF16)

        for p in range(NPAIR):
            bs = [2 * p, 2 * p, 2 * p + 1, 2 * p + 1]
            tch = [0, 1, 0, 1]

            def batched_rstd(mv8):
                """[128,8] interleaved (mean,var) x4 -> rstd4 [128,4]."""
                mv_v = mv8.rearrange("p (i two) -> p i two", two=2)
                var4 = mv_v[:, :, 1]
                vpe = st.tile([128, 4], F32, name="vpe", tag="vpe")
                nc.vector.tensor_scalar(
                    out=vpe, in0=var4, scalar1=EPS, scalar2=None, op0=ALU.add
                )
                s0h = st.tile([128, 4], I32, name="s0h", tag="s0h")
                nc.vector.tensor_scalar(
                    out=s0h, in0=vpe.bitcast(I32), scalar1=1, scalar2=None,
                    op0=ALU.logical_shift_right,
                )
                s0i = st.tile([128, 4], I32, name="s0i", tag="s0i")
                nc.vector.tensor_scalar(
                    out=s0i, in0=s0h, scalar1=SQRT_MAGIC, scalar2=None,
                    op0=ALU.add,
                )
                cur = s0i.bitcast(F32)
                for hi in range(2):
                    r_ = st.tile([128, 4], F32, name=f"hr{hi}", tag=f"hr{hi}")
                    nc.vector.reciprocal(r_, cur)
                    t_ = st.tile([128, 4], F32, name=f"ht{hi}", tag=f"ht{hi}")
                    nc.vector.tensor_mul(t_, vpe, r_)
                    s_ = st.tile([128, 4], F32, name=f"hs{hi}", tag=f"hs{hi}")
                    nc.vector.tensor_add(s_, t_, cur)
                    sh = st.tile([128, 4], F32, name=f"hh{hi}", tag=f"hh{hi}")
                    nc.vector.tensor_scalar_mul(sh, s_, 0.5)
                    cur = sh
                rstd4 = st.tile([128, 4], F32, name="rstd4", tag="rstd4")
                nc.vector.reciprocal(rstd4, cur)
                return rstd4

            def layernorm4(dsts, srcs, g_sb, b_sb):
                mv8 = st.tile([128, 8], F32, name="mv8", tag="mv8")
                for i in range(4):
                    stats = st.tile([128, 6], F32, name="lst", tag="lst")
                    nc.vector.bn_stats(stats, srcs[i])
                    nc.vector.bn_aggr(mv8[:, 2 * i : 2 * i + 2], stats)
                rstd4 = batched_rstd(mv8)
                # (x - mu) * rstd == x * rstd + (-mu * rstd): affine -> ACT
                mv_v = mv8.rearrange("p (i two) -> p i two", two=2)
                negmr = st.tile([128, 4], F32, name="negmr", tag="negmr")
                nc.vector.scalar_tensor_tensor(
                    out=negmr, in0=mv_v[:, :, 0], scalar=-1.0, in1=rstd4,
                    op0=ALU.mult, op1=ALU.mult,
                )
                for i in range(4):
                    nc.scalar.activation(
                        out=dsts[i], in_=srcs[i],
                        func=mybir.ActivationFunctionType.Identity,
                        bias=negmr[:, i : i + 1],
                        scale=rstd4[:, i : i + 1],
                    )
                    if g_sb is not None:
                        nc.vector.tensor_mul(dsts[i], dsts[i], g_sb)
                    if b_sb is not None:
                        nc.vector.tensor_add(dsts[i], dsts[i], b_sb)

            def transpose_into(dstT, src, i):
                for c in range(KC):
                    pt = ps.tile([128, 128], BF16, name="pa", tag="pa")
                    nc.tensor.transpose(
                        pt, src[:, c * 128 : (c + 1) * 128], id_sb
                    )
                    nc.scalar.copy(dstT[c][:, i * 128 : (i + 1) * 128], pt)

            # ---- stage 1: load x, LN1, transpose -> hT (bf16, C-major) ----
            xt = [
                sb.tile([128, C], F32, name=f"xt{i}", tag=f"xt{i}", bufs=2)
                for i in range(4)
            ]
            for i in range(4):
                nc.sync.dma_start(
                    out=xt[i],
                    in_=x[bs[i], tch[i] * 128 : (tch[i] + 1) * 128, :],
                )
            hT = [
                sb.tile([128, 2 * T], BF16, name=f"hT{c}", tag=f"hT{c}", bufs=2)
                for c in range(KC)
            ]
            ht_ = [
                sb.tile([128, C], BF16, name=f"h{i}", tag=f"h{i}", bufs=2)
                for i in range(4)
            ]
            layernorm4(ht_, xt, g1_sb, b1ln_sb)
            for i in range(4):
                transpose_into(hT, ht_[i], i)

            # ---- stage 2: q^T, k^T (C-major bf16), v (token-major f32r) ----
            qT = [
                sb.tile([128, 2 * T], BF16, name=f"qT{m}", tag=f"qT{m}", bufs=2)
                for m in range(KC)
            ]
            kT = [
                sb.tile([128, 2 * T], BF16, name=f"kT{m}", tag=f"kT{m}", bufs=2)
                for m in range(KC)
            ]
            for m in range(KC):
                pq = ps.tile([128, 2 * T], F32, name="pa", tag="pa")
                for k in range(KC):
                    nc.tensor.matmul(
                        pq, wq_sb[k][:, m * 128 : (m + 1) * 128], hT[k],
                        start=(k == 0), stop=(k == KC - 1),
                    )
                nc.scalar.copy(qT[m], pq)
                pk = ps.tile([128, 2 * T], F32, name="pa", tag="pa")
                for k in range(KC):
                    nc.tensor.matmul(
                        pk, wk_sb[k][:, m * 128 : (m + 1) * 128], hT[k],
                        start=(k == 0), stop=(k == KC - 1),
                    )
                nc.scalar.copy(kT[m], pk)
            vt = [
                sb.tile([128, C], BF16, name=f"v{i}", tag=f"v{i}", bufs=2)
                for i in range(4)
            ]
            for i in range(4):
                pv = ps.tile([128, C], F32, name="pa", tag="pa")
                for k in range(KC):
                    nc.tensor.matmul(
                        pv, hT[k][:, i * 128 : (i + 1) * 128], wv_sb[k],
                        start=(k == 0), stop=(k == KC - 1),
                    )
                nc.scalar.copy(vt[i], pv)

            # ---- stage 3: attention, S^T orientation ----
            acT = [
                sb.tile([128, 2 * T], BF16, name=f"acT{c}", tag=f"acT{c}", bufs=2)
                for c in range(KC)
            ]
            for ib in range(2):
                q0 = ib * T
                for ch in range(KC):
                    eT = []
                    for par in range(2):
                        off = par * 64
                        # scores: S^T[key, query]; kc0 block cols 0:256
                        # (keys 0:128 x all q), kc1 block cols 256:512
                        # (keys 128:256 x all q, q<128 masked)
                        pS = ps.tile(
                            [128, 2 * T], F32, name="pa", tag="pa"
                        )
                        for kc in range(2):
                            nc.tensor.matmul(
                                pS[:, kc * T : (kc + 1) * T],
                                kT[ch][
                                    off : off + 64,
                                    q0 + kc * 128 : q0 + (kc + 1) * 128,
                                ],
                                qT[ch][off : off + 64, q0 : q0 + T],
                                start=True, stop=True,
                                tile_position=(off, 0),
                            )
                        # exp + causal mask fused: rint(s*EXP_S + maskbias)
                        e_ = sb.tile(
                            [128, 2 * T], I16,
                            name=f"eT{par}", tag=f"eT{par}", bufs=2,
                        )
                        nc.vector.scalar_tensor_tensor(
                            out=e_, in0=pS, scalar=EXP_S, in1=mb_sb,
                            op0=ALU.mult, op1=ALU.add,
                        )
                        eT.append(e_.bitcast(BF16))

                    # rowsums: ones-column matmuls, both heads -> one bank
                    rs = ps.tile([1, 2 * T], F32, name="pa", tag="pa")
                    for par in range(2):
                        for kc in range(2):
                            nc.tensor.matmul(
                                rs[0:1, par * T : (par + 1) * T],
                                onescol_sb,
                                eT[par][:, kc * T : (kc + 1) * T],
                                start=(kc == 0), stop=(kc == 1),
                            )
                    # raw rowsums -> SBUF (bf16) so the PE can broadcast them;
                    # the reciprocal runs AFTER the broadcast so it is
                    # partition-parallel (DVE reciprocal costs ~4 cyc per
                    # FREE-dim element; a [1,512] row recip is 2.1us, a
                    # [128,256] stacked recip is 1.1us for 2 head-batches)
                    rsS = sb.tile([1, 2 * T], BF16, name="rsS", tag="rsS",
                                  bufs=3)
                    nc.scalar.copy(rsS, rs)

                    if COLTILE:
                        # attn out + rowsum broadcast, heads stacked in one
                        # bank via col-tiling
                        pA = ps.tile([128, T], F32, name="pa", tag="pa")
                        rsB = ps.tile([128, T], F32, name="pa", tag="pa")
                        for par in range(2):
                            off = par * 64
                            hh = 2 * ch + par
                            for kc in range(2):
                                nc.tensor.matmul(
                                    pA[off : off + 64, :],
                                    vt[ib * 2 + kc][:, hh * 64 : hh * 64 + 64],
                                    eT[par][:, kc * T : (kc + 1) * T],
                                    start=(kc == 0), stop=(kc == 1),
                                    tile_position=(0, off),
                                )
                            nc.tensor.matmul(
                                rsB[off : off + 64, :],
                                ones1r_sb[0:1, 0:64],
                                rsS[0:1, par * T : (par + 1) * T],
                                start=True, stop=True,
                                tile_position=(0, off),
                            )
                        rrS = sb.tile([128, T], BF16, name="rrS", tag="rrS",
                                      bufs=3)
                        with nc.allow_low_precision("bf16 1/rowsum, ok"):
                            nc.vector.reciprocal(rrS, rsB)
                        nc.vector.tensor_mul(
                            acT[ch][:, q0 : q0 + T], pA, rrS
                        )
                    else:
                        # fallback: full-height matmuls (garbage in the other
                        # head's rows, never read), no col-tiling needed
                        for par in range(2):
                            o_ = par * 64
                            pA = ps.tile([128, T], F32, name="pa", tag="pa")
                            for kc in range(2):
                                nc.tensor.matmul(
                                    pA,
                                    vt[ib * 2 + kc][:, ch * 128 : (ch + 1) * 128],
                                    eT[par][:, kc * T : (kc + 1) * T],
                                    start=(kc == 0), stop=(kc == 1),
                                )
                            rsB = ps.tile([128, T], F32, name="pa", tag="pa")
                            nc.tensor.matmul(
                                rsB, ones1r_sb,
                                rsS[0:1, par * T : (par + 1) * T],
                                start=True, stop=True,
                            )
                            rrS = sb.tile([128, T], BF16, name="rrS",
                                          tag="rrS", bufs=3)
                            with nc.allow_low_precision("bf16 1/rowsum, ok"):
                                nc.vector.reciprocal(rrS, rsB)
                            nc.vector.tensor_mul(
                                acT[ch][o_ : o_ + 64, q0 : q0 + T],
                                pA[o_ : o_ + 64, :],
                                rrS[o_ : o_ + 64, :],
                            )

            # ---- stage 4: proj + residual -> y ----
            yt = [
                sb.tile([128, C], F32, name=f"y{i}", tag=f"y{i}", bufs=2)
                for i in range(4)
            ]
            for i in range(4):
                pP = ps.tile([128, C], F32, name="pa", tag="pa")
                for k in range(KC):
                    nc.tensor.matmul(
                        pP, acT[k][:, i * 128 : (i + 1) * 128], wp_sb[k],
                        start=(k == 0), stop=(k == KC - 1),
                    )
                nc.vector.tensor_add(yt[i], pP, xt[i])
                if bpb_sb is not None:
                    nc.vector.tensor_add(yt[i], yt[i], bpb_sb)

            # ---- stage 5: LN2 + transpose -> h2T ----
            h2T = [
                sb.tile([128, 2 * T], BF16, name=f"h2T{c}", tag=f"h2T{c}", bufs=2)
                for c in range(KC)
            ]
            h2_ = [
                sb.tile([128, C], BF16, name=f"h2{i}", tag=f"h2{i}", bufs=2)
                for i in range(4)
            ]
            layernorm4(h2_, yt, g2_sb, b2ln_sb)
            for i in range(4):
                transpose_into(h2T, h2_[i], i)

            # ---- stage 6: MLP ----
            m1r = [
                sb.tile([128, 2 * T], BF16, name=f"m1r{m}", tag=f"m1r{m}", bufs=2)
                for m in range(KH)
            ]
            for m in range(KH):
                pM = ps.tile([128, 2 * T], F32, name="pa", tag="pa")
                for k in range(KC):
                    nc.tensor.matmul(
                        pM, w1_sb[k][:, m * 128 : (m + 1) * 128], h2T[k],
                        start=(k == 0), stop=(k == KC - 1),
                    )
                nc.scalar.activation(
                    out=m1r[m], in_=pM,
                    func=mybir.ActivationFunctionType.Relu,
                    bias=(b1c_sb[:, m : m + 1] if use_b1 else 0.0),
                    scale=1.0,
                )

            # ---- stage 6b/7: MLP2 (one PSUM bank at a time) + residual ----
            for i in range(4):
                psY = ps.tile([128, C], F32, name="pa", tag="pa")
                for m in range(KH):
                    nc.tensor.matmul(
                        psY, m1r[m][:, i * 128 : (i + 1) * 128], w2_sb[m],
                        start=(m == 0), stop=(m == KH - 1),
                    )
                ot = sb.tile([128, C], F32, name=f"ot{i}", tag=f"ot{i}", bufs=2)
                nc.vector.tensor_add(ot, psY, yt[i])
                if b2b_sb is not None:
                    nc.vector.tensor_add(ot, ot, b2b_sb)
                nc.sync.dma_start(
                    out=out[bs[i], tch[i] * 128 : (tch[i] + 1) * 128, :],
                    in_=ot,
                )
    nc.compile()
    return nc


def _to_bf16(a):
    import ml_dtypes

    return np.ascontiguousarray(a.astype(ml_dtypes.bfloat16))


def _host_prep(inputs):
    f = np.float32
    x = np.ascontiguousarray(inputs["x"], dtype=f)
    wq_full = (
        np.asarray(inputs["wq"], dtype=f).transpose(1, 0, 2).reshape(C, C)
        * (C ** -0.5)
    )
    wk_full = np.asarray(inputs["wk"], dtype=f).transpose(1, 0, 2).reshape(C, C)
    wv_full = np.asarray(inputs["wv"], dtype=f).transpose(1, 0, 2).reshape(C, C)
    wp_ = np.asarray(inputs["w_proj"], dtype=f)
    w1_ = np.asarray(inputs["w1"], dtype=f)
    w2_ = np.asarray(inputs["w2"], dtype=f)
    tile128 = lambda v: np.ascontiguousarray(
        np.broadcast_to(np.asarray(v, dtype=f), (128, C))
    )
    g1 = tile128(inputs["ln1_g"])
    b1ln = tile128(inputs["ln1_b"])
    g2 = tile128(inputs["ln2_g"])
    b2ln = tile128(inputs["ln2_b"])
    bpb = tile128(inputs["b_proj"])
    b2b = tile128(inputs["b2"])
    b1c = np.ascontiguousarray(np.asarray(inputs["b1"], dtype=f).reshape(KH, 128).T)

    # exp maskbias: [128, 512]; cols 0:256 = keys 0:128 (allow q >= s),
    # cols 256:512 = keys 128:256 (allow q >= 128 + s)
    s_idx = np.arange(128)[:, None]
    q_idx = np.arange(T)[None, :]
    mb0 = np.where(q_idx >= s_idx, EXP_ALLOW, EXP_MASKED)
    mb1 = np.where(q_idx >= 128 + s_idx, EXP_ALLOW, EXP_MASKED)
    mbias = np.ascontiguousarray(
        np.concatenate([mb0, mb1], axis=1).astype(f)
    )
    import ml_dtypes

    identb = np.ascontiguousarray(np.eye(128, dtype=ml_dtypes.bfloat16))
    ones1r = np.ones((1, 128), dtype=ml_dtypes.bfloat16)
    onescol = np.ones((128, 1), dtype=ml_dtypes.bfloat16)

    flags = (
        bool(not np.all(np.asarray(inputs["ln1_g"]) == 1.0)),
        bool(np.any(np.asarray(inputs["ln1_b"]))),
        bool(not np.all(np.asarray(inputs["ln2_g"]) == 1.0)),
        bool(np.any(np.asarray(inputs["ln2_b"]))),
        bool(np.any(np.asarray(inputs["b_proj"]))),
        bool(np.any(np.asarray(inputs["b1"]))),
        bool(np.any(np.asarray(inputs["b2"]))),
    )
    shared = dict(
        wq=_to_bf16(wq_full), wk=_to_bf16(wk_full), wv=_to_bf16(wv_full),
        wp=_to_bf16(wp_), w1=_to_bf16(w1_), w2=_to_bf16(w2_),
        g1=g1, b1ln=b1ln, g2=g2, b2ln=b2ln, bpb=bpb, b2b=b2b, b1c=b1c,
        mb=mbias, identb=identb, ones1r=ones1r, onescol=onescol,
    )
    in_maps = []
    for i in range(NCORES):
        m = dict(shared)
        m["x"] = np.ascontiguousarray(x[i * BL : (i + 1) * BL])
        in_maps.append(m)
    return in_maps, flags


_NC_CACHE = {}


def _get_program(flags):
    if flags not in _NC_CACHE:
        _NC_CACHE[flags] = build_program(*flags)
    return _NC_CACHE[flags]


def run(inputs, **spmd_kwargs):
    from concourse.bass_utils import run_bass_kernel_spmd

    in_maps, flags = _host_prep(inputs)
    nc = _get_program(flags)
    bkr = run_bass_kernel_spmd(nc, in_maps, list(range(NCORES)), **spmd_kwargs)
    outs = [bkr.results[i]["out"] for i in range(NCORES)]
    return np.concatenate(outs, axis=0).astype(np.float32), bkr


def kernel(**inputs):
    full, _ = run(inputs)
    return full


# revision 6
# speedup vs baseline: 2.6319x; 1.4260x over previous
"""Trainium2 Bass kernel for a dense transformer block (B=128, T=256, C=384, H=6).

Sharding: data-parallel over batch across 8 NeuronCores (16 batches/core),
identical SPMD program per core, no collectives.

Design (v4):
  - Attention computed in transposed orientation: scores are produced as
    S^T[key, query] directly (lhsT=k, rhs=q), so the softmax weights feed the
    attention matmul (lhsT=v, rhs=expS^T) with NO per-tile PE transposes and
    NO scalar-engine copies of the weights.
  - Softmax: exp + causal mask fused into ONE DVE op per score tile via an
    int16 Schraudolph (bf16 shares f32's 8-bit exponent):
    e = bitcast_bf16(int16(rint(s * 2^7/ln2 + maskbias))), maskbias holding
    127*2^7 for allowed entries and 40*2^7 (-> ~1e-26) for masked ones.
    End-to-end rel err ~9e-3 vs the 2e-2 budget.
  - Softmax denominators via ones-column matmuls on the PE (sum over keys is
    a partition-dim reduction); raw rowsums are PE-broadcast across
    partitions (tiny K=1 matmul), reciprocal runs partition-parallel on the
    broadcast (DVE reciprocal costs ~4 cyc/free-elem; approx_fast ~5x less),
    and the normalize multiply doubles as the PSUM->SBUF move of the
    attention output.
  - All matmuls bf16 (weights converted host-side; activations converted in
    the PSUM->SBUF copies). fp32 residual spine.
  - LayerNorm: stats on DVE (bn_stats), apply as x*rstd + (-mu*rstd) on the
    ACT engine (1-bucket affine table, same class as Copy). Relu on ACT.
  - Head pairs packed: score matmuls row-tiled (two K=64 heads concurrent in
    row groups 0-1/2-3), attention + broadcast matmuls col-tiled to stack the
    two heads into partitions 0:64 / 64:128 of one PSUM bank.
  - Software-pipelined emission: the in-order engine queues are laid out so
    no engine head-of-line-blocks: next pair's DMA+LN1 is emitted before the
    current pair's MLP; attention units pipeline scores(u+1) ahead of
    attn(u), and bcast/recip/normalize(u-1) behind attn(u).
"""

import numpy as np

import concourse.bass as bass
import concourse.mybir as mybir
from concourse import bacc
from concourse.tile import TileContext
from contextlib import ExitStack

B, T, C = 128, 256, 384
H, D = 6, 64
FF = 4 * C
NCORES = 8
BL = B // NCORES  # 16
NPAIR = BL // 2  # 8
KC = C // 128  # 3
KH = FF // 128  # 12
EPS = 1e-5
F32 = mybir.dt.float32
BF16 = mybir.dt.bfloat16
I32 = mybir.dt.int32
I16 = mybir.dt.int16
ALU = mybir.AluOpType
AF = mybir.ActivationFunctionType

EXP_S = float(2**7 / np.log(2.0))
EXP_ALLOW = float(127 * 2**7)
EXP_MASKED = float(40 * 2**7)
SQRT_MAGIC = 0x1FBD1DF5


def build_program(use_g1, use_b1ln, use_g2, use_b2ln, use_bp, use_b1, use_b2):
    nc = bacc.Bacc(None)
    x = nc.declare_dram_parameter("x", [BL, T, C], F32, isOutput=False)
    wq = nc.declare_dram_parameter("wq", [C, C], BF16, isOutput=False)
    wk = nc.declare_dram_parameter("wk", [C, C], BF16, isOutput=False)
    wv = nc.declare_dram_parameter("wv", [C, C], BF16, isOutput=False)
    wp = nc.declare_dram_parameter("wp", [C, C], BF16, isOutput=False)
    w1 = nc.declare_dram_parameter("w1", [C, FF], BF16, isOutput=False)
    w2 = nc.declare_dram_parameter("w2", [FF, C], BF16, isOutput=False)
    g1 = nc.declare_dram_parameter("g1", [128, C], F32, isOutput=False)
    b1ln = nc.declare_dram_parameter("b1ln", [128, C], F32, isOutput=False)
    g2 = nc.declare_dram_parameter("g2", [128, C], F32, isOutput=False)
    b2ln = nc.declare_dram_parameter("b2ln", [128, C], F32, isOutput=False)
    bpb = nc.declare_dram_parameter("bpb", [128, C], F32, isOutput=False)
    b2b = nc.declare_dram_parameter("b2b", [128, C], F32, isOutput=False)
    b1c = nc.declare_dram_parameter("b1c", [128, KH], F32, isOutput=False)
    mb = nc.declare_dram_parameter("mb", [128, 2 * T], F32, isOutput=False)
    identb = nc.declare_dram_parameter("identb", [128, 128], BF16, isOutput=False)
    ones1r = nc.declare_dram_parameter("ones1r", [1, 128], BF16, isOutput=False)
    onescol = nc.declare_dram_parameter("onescol", [128, 1], BF16, isOutput=False)
    out = nc.declare_dram_parameter("out", [BL, T, C], F32, isOutput=True)

    with TileContext(nc) as tc, ExitStack() as ctx:
        wts = ctx.enter_context(tc.tile_pool(name="wts", bufs=1))
        sb = ctx.enter_context(tc.tile_pool(name="sb", bufs=1))
        st = ctx.enter_context(tc.tile_pool(name="st", bufs=4))
        ps = ctx.enter_context(tc.tile_pool(name="ps", bufs=8, space="PSUM"))

        def load_chunks(dram, n, width, tagp, dt):
            tiles = []
            for k in range(n):
                t_ = wts.tile([128, width], dt, name=f"{tagp}{k}", tag=f"{tagp}{k}")
                nc.sync.dma_start(out=t_, in_=dram[k * 128 : (k + 1) * 128, :])
                tiles.append(t_)
            return tiles

        wq_sb = load_chunks(wq, KC, C, "wq", BF16)
        wk_sb = load_chunks(wk, KC, C, "wk", BF16)
        wv_sb = load_chunks(wv, KC, C, "wv", BF16)
        wp_sb = load_chunks(wp, KC, C, "wp", BF16)
        w1_sb = load_chunks(w1, KC, FF, "w1", BF16)
        w2_sb = load_chunks(w2, KH, C, "w2", BF16)

        def load_one(dram, shape, tag, dt=F32):
            t_ = wts.tile(shape, dt, name=tag, tag=tag)
            nc.sync.dma_start(out=t_, in_=dram[:, :])
            return t_

        g1_sb = load_one(g1, [128, C], "g1") if use_g1 else None
        b1ln_sb = load_one(b1ln, [128, C], "b1ln") if use_b1ln else None
        g2_sb = load_one(g2, [128, C], "g2") if use_g2 else None
        b2ln_sb = load_one(b2ln, [128, C], "b2ln") if use_b2ln else None
        bpb_sb = load_one(bpb, [128, C], "bpb") if use_bp else None
        b2b_sb = load_one(b2b, [128, C], "b2b") if use_b2 else None
        b1c_sb = load_one(b1c, [128, KH], "b1c") if use_b1 else None
        mb_sb = load_one(mb, [128, 2 * T], "mb")
        id_sb = load_one(identb, [128, 128], "identb", BF16)
        ones1r_sb = load_one(ones1r, [1, 128], "ones1r", BF16)
        onescol_sb = load_one(onescol, [128, 1], "onescol", BF16)

        def batched_rstd(mv8):
            """[128,8] interleaved (mean,var) x4 -> rstd4 [128,4]."""
            mv_v = mv8.rearrange("p (i two) -> p i two", two=2)
            var4 = mv_v[:, :, 1]
            vpe = st.tile([128, 4], F32, name="vpe", tag="vpe")
            nc.vector.tensor_scalar(
                out=vpe, in0=var4, scalar1=EPS, scalar2=None, op0=ALU.add
            )
            s0h = st.tile([128, 4], I32, name="s0h", tag="s0h")
            nc.vector.tensor_scalar(
                out=s0h, in0=vpe.bitcast(I32), scalar1=1, scalar2=None,
                op0=ALU.logical_shift_right,
            )
            s0i = st.tile([128, 4], I32, name="s0i", tag="s0i")
            nc.vector.tensor_scalar(
                out=s0i, in0=s0h, scalar1=SQRT_MAGIC, scalar2=None, op0=ALU.add
            )
            cur = s0i.bitcast(F32)
            for hi in range(2):
                r_ = st.tile([128, 4], F32, name=f"hr{hi}", tag=f"hr{hi}")
                nc.vector.reciprocal(r_, cur)
                t_ = st.tile([128, 4], F32, name=f"ht{hi}", tag=f"ht{hi}")
                nc.vector.tensor_mul(t_, vpe, r_)
                s_ = st.tile([128, 4], F32, name=f"hs{hi}", tag=f"hs{hi}")
                nc.vector.tensor_add(s_, t_, cur)
                sh = st.tile([128, 4], F32, name=f"hh{hi}", tag=f"hh{hi}")
                nc.vector.tensor_scalar_mul(sh, s_, 0.5)
                cur = sh
            rstd4 = st.tile([128, 4], F32, name="rstd4", tag="rstd4")
            nc.vector.reciprocal(rstd4, cur)
            return rstd4

        def layernorm4(dsts, srcs, g_sb, b_sb):
            mv8 = st.tile([128, 8], F32, name="mv8", tag="mv8")
            for i in range(4):
                stats = st.tile([128, 6], F32, name="lst", tag="lst")
                nc.vector.bn_stats(stats, srcs[i])
                nc.vector.bn_aggr(mv8[:, 2 * i : 2 * i + 2], stats)
            rstd4 = batched_rstd(mv8)
            # (x - mu) * rstd == x * rstd + (-mu * rstd): affine -> ACT
            mv_v = mv8.rearrange("p (i two) -> p i two", two=2)
            negmr = st.tile([128, 4], F32, name="negmr", tag="negmr")
            nc.vector.scalar_tensor_tensor(
                out=negmr, in0=mv_v[:, :, 0], scalar=-1.0, in1=rstd4,
                op0=ALU.mult, op1=ALU.mult,
            )
            for i in range(4):
                nc.scalar.activation(
                    out=dsts[i], in_=srcs[i], func=AF.Identity,
                    bias=negmr[:, i : i + 1], scale=rstd4[:, i : i + 1],
                )
                if g_sb is not None:
                    nc.vector.tensor_mul(dsts[i], dsts[i], g_sb)
                if b_sb is not None:
                    nc.vector.tensor_add(dsts[i], dsts[i], b_sb)

        def transpose_into(dstT, src, i):
            for c in range(KC):
                pt = ps.tile([128, 128], BF16, name="pa", tag="pa")
                nc.tensor.transpose(pt, src[:, c * 128 : (c + 1) * 128], id_sb)
                nc.any.tensor_copy(dstT[c][:, i * 128 : (i + 1) * 128], pt)

        def stage1_ln(p):
            """DMA x, LN1 -> ht_ (bf16). Returns pair state dict."""
            bs = [2 * p, 2 * p, 2 * p + 1, 2 * p + 1]
            tch = [0, 1, 0, 1]
            S = {"bs": bs, "tch": tch}
            S["xt"] = [
                sb.tile([128, C], F32, name=f"xt{i}", tag=f"xt{i}", bufs=2)
                for i in range(4)
            ]
            for i in range(4):
                nc.sync.dma_start(
                    out=S["xt"][i],
                    in_=x[bs[i], tch[i] * 128 : (tch[i] + 1) * 128, :],
                )
            S["ht"] = [
                sb.tile([128, C], BF16, name=f"h{i}", tag=f"h{i}", bufs=2)
                for i in range(4)
            ]
            layernorm4(S["ht"], S["xt"], g1_sb, b1ln_sb)
            return S

        def stage1_t(S):
            S["hT"] = [
                sb.tile([128, 2 * T], BF16, name=f"hT{c}", tag=f"hT{c}", bufs=2)
                for c in range(KC)
            ]
            for i in range(4):
                transpose_into(S["hT"], S["ht"][i], i)

        def stage2(S):
            hT = S["hT"]
            S["qT"] = [
                sb.tile([128, 2 * T], BF16, name=f"qT{m}", tag=f"qT{m}", bufs=2)
                for m in range(KC)
            ]
            S["kT"] = [
                sb.tile([128, 2 * T], BF16, name=f"kT{m}", tag=f"kT{m}", bufs=2)
                for m in range(KC)
            ]
            for m in range(KC):
                pq = ps.tile([128, 2 * T], F32, name="pa", tag="pa")
                for k in range(KC):
                    nc.tensor.matmul(
                        pq, wq_sb[k][:, m * 128 : (m + 1) * 128], hT[k],
                        start=(k == 0), stop=(k == KC - 1),
                    )
                nc.scalar.copy(S["qT"][m], pq)
                pk = ps.tile([128, 2 * T], F32, name="pa", tag="pa")
                for k in range(KC):
                    nc.tensor.matmul(
                        pk, wk_sb[k][:, m * 128 : (m + 1) * 128], hT[k],
                        start=(k == 0), stop=(k == KC - 1),
                    )
                nc.scalar.copy(S["kT"][m], pk)
            S["vt"] = [
                sb.tile([128, C], BF16, name=f"v{i}", tag=f"v{i}", bufs=2)
                for i in range(4)
            ]
            for i in range(4):
                pv = ps.tile([128, C], F32, name="pa", tag="pa")
                for k in range(KC):
                    nc.tensor.matmul(
                        pv, hT[k][:, i * 128 : (i + 1) * 128], wv_sb[k],
                        start=(k == 0), stop=(k == KC - 1),
                    )
                nc.any.tensor_copy(S["vt"][i], pv)

        def attention(S):
            """Software-pipelined: scores/exp one unit ahead of the attn
            matmuls; bcast/recip/normalize one unit behind."""
            qT, kT, vt = S["qT"], S["kT"], S["vt"]
            S["acT"] = [
                sb.tile([128, 2 * T], BF16, name=f"acT{c}", tag=f"acT{c}",
                        bufs=2)
                for c in range(KC)
            ]
            units = [(ib, ch) for ib in range(2) for ch in range(KC)]
            ust = [dict() for _ in units]

            def phase_a(u):
                ib, ch = units[u]
                q0 = ib * T
                eT = []
                for par in range(2):
                    off = par * 64
                    pS = ps.tile([128, 2 * T], F32, name="pa", tag="pa")
                    for kc in range(2):
                        nc.tensor.matmul(
                            pS[:, kc * T : (kc + 1) * T],
                            kT[ch][
                                off : off + 64,
                                q0 + kc * 128 : q0 + (kc + 1) * 128,
                            ],
                            qT[ch][off : off + 64, q0 : q0 + T],
                            start=True, stop=True,
                            tile_position=(off, 0),
                        )
                    e_ = sb.tile(
                        [128, 2 * T], I16,
                        name=f"eT{par}", tag=f"eT{par}", bufs=3,
                    )
                    nc.vector.scalar_tensor_tensor(
                        out=e_, in0=pS, scalar=EXP_S, in1=mb_sb,
                        op0=ALU.mult, op1=ALU.add,
                    )
                    eT.append(e_.bitcast(BF16))
                ust[u]["eT"] = eT

            def phase_c(u):
                ib, ch = units[u]
                eT = ust[u]["eT"]
                pA = ps.tile([128, T], F32, name="pa", tag="pa")
                rs = ps.tile([1, 2 * T], F32, name="pa", tag="pa")
                for par in range(2):
                    off = par * 64
                    hh = 2 * ch + par
                    for kc in range(2):
                        nc.tensor.matmul(
                            pA[off : off + 64, :],
                            vt[ib * 2 + kc][:, hh * 64 : hh * 64 + 64],
                            eT[par][:, kc * T : (kc + 1) * T],
                            start=(kc == 0), stop=(kc == 1),
                            tile_position=(0, off),
                        )
                    for kc in range(2):
                        nc.tensor.matmul(
                            rs[0:1, par * T : (par + 1) * T],
                            onescol_sb,
                            eT[par][:, kc * T : (kc + 1) * T],
                            start=(kc == 0), stop=(kc == 1),
                        )
                rsS = sb.tile([1, 2 * T], BF16, name="rsS", tag="rsS", bufs=3)
                nc.scalar.copy(rsS, rs)
                ust[u]["pA"], ust[u]["rsS"] = pA, rsS

            def phase_b(u):
                ib, ch = units[u]
                q0 = ib * T
                pA, rsS = ust[u]["pA"], ust[u]["rsS"]
                rsB = ps.tile([128, T], F32, name="pa", tag="pa")
                for par in range(2):
                    off = par * 64
                    nc.tensor.matmul(
                        rsB[off : off + 64, :],
                        ones1r_sb[0:1, 0:64],
                        rsS[0:1, par * T : (par + 1) * T],
                        start=True, stop=True,
                        tile_position=(0, off),
                    )
                rrS = sb.tile([128, T], F32, name="rrS", tag="rrS", bufs=3)
                nc.vector.reciprocal_approx_fast(out=rrS, in_=rsB)
                nc.vector.tensor_mul(S["acT"][ch][:, q0 : q0 + T], pA, rrS)

            phase_a(0)
            for u in range(len(units)):
                if u + 1 < len(units):
                    phase_a(u + 1)
                phase_c(u)
                if u >= 1:
                    phase_b(u - 1)
            phase_b(len(units) - 1)

        def proj_resid(S):
            S["yt"] = [
                sb.tile([128, C], F32, name=f"y{i}", tag=f"y{i}", bufs=2)
                for i in range(4)
            ]
            for i in range(4):
                pP = ps.tile([128, C], F32, name="pa", tag="pa")
                for k in range(KC):
                    nc.tensor.matmul(
                        pP, S["acT"][k][:, i * 128 : (i + 1) * 128], wp_sb[k],
                        start=(k == 0), stop=(k == KC - 1),
                    )
                nc.vector.tensor_add(S["yt"][i], pP, S["xt"][i])
                if bpb_sb is not None:
                    nc.vector.tensor_add(S["yt"][i], S["yt"][i], bpb_sb)

        def ln2_t(S):
            S["h2T"] = [
                sb.tile([128, 2 * T], BF16, name=f"h2T{c}", tag=f"h2T{c}",
                        bufs=2)
                for c in range(KC)
            ]
            h2_ = [
                sb.tile([128, C], BF16, name=f"h2{i}", tag=f"h2{i}", bufs=2)
                for i in range(4)
            ]
            layernorm4(h2_, S["yt"], g2_sb, b2ln_sb)
            for i in range(4):
                transpose_into(S["h2T"], h2_[i], i)

        def mlp_store(S):
            m1r = [
                sb.tile([128, 2 * T], BF16, name=f"m1r{m}", tag=f"m1r{m}",
                        bufs=2)
                for m in range(KH)
            ]
            for m in range(KH):
                pM = ps.tile([128, 2 * T], F32, name="pa", tag="pa")
                for k in range(KC):
                    nc.tensor.matmul(
                        pM, w1_sb[k][:, m * 128 : (m + 1) * 128], S["h2T"][k],
                        start=(k == 0), stop=(k == KC - 1),
                    )
                nc.scalar.activation(
                    out=m1r[m], in_=pM, func=AF.Relu,
                    bias=(b1c_sb[:, m : m + 1] if use_b1 else 0.0),
                    scale=1.0,
                )
            for i in range(4):
                psY = ps.tile([128, C], F32, name="pa", tag="pa")
                for m in range(KH):
                    nc.tensor.matmul(
                        psY, m1r[m][:, i * 128 : (i + 1) * 128], w2_sb[m],
                        start=(m == 0), stop=(m == KH - 1),
                    )
                ot = sb.tile([128, C], F32, name=f"ot{i}", tag=f"ot{i}", bufs=2)
                nc.vector.tensor_add(ot, psY, S["yt"][i])
                if b2b_sb is not None:
                    nc.vector.tensor_add(ot, ot, b2b_sb)
                nc.sync.dma_start(
                    out=out[
                        S["bs"][i], S["tch"][i] * 128 : (S["tch"][i] + 1) * 128, :
                    ],
                    in_=ot,
                )

        # --- software-pipelined pair loop ---
        S = stage1_ln(0)
        stage1_t(S)
        nextS = None
        for p in range(NPAIR):
            stage2(S)
            attention(S)
            if p + 1 < NPAIR:
                nextS = stage1_ln(p + 1)
            proj_resid(S)
            if p + 1 < NPAIR:
                stage1_t(nextS)
            ln2_t(S)
            mlp_store(S)
            S = nextS
    nc.compile()
    return nc


def _to_bf16(a):
    import ml_dtypes

    return np.ascontiguousarray(a.astype(ml_dtypes.bfloat16))


def _host_prep(inputs):
    f = np.float32
    x = np.ascontiguousarray(inputs["x"], dtype=f)
    wq_full = (
        np.asarray(inputs["wq"], dtype=f).transpose(1, 0, 2).reshape(C, C)
        * (C ** -0.5)
    )
    wk_full = np.asarray(inputs["wk"], dtype=f).transpose(1, 0, 2).reshape(C, C)
    wv_full = np.asarray(inputs["wv"], dtype=f).transpose(1, 0, 2).reshape(C, C)
    wp_ = np.asarray(inputs["w_proj"], dtype=f)
    w1_ = np.asarray(inputs["w1"], dtype=f)
    w2_ = np.asarray(inputs["w2"], dtype=f)
    tile128 = lambda v: np.ascontiguousarray(
        np.broadcast_to(np.asarray(v, dtype=f), (128, C))
    )
    g1 = tile128(inputs["ln1_g"])
    b1ln = tile128(inputs["ln1_b"])
    g2 = tile128(inputs["ln2_g"])
    b2ln = tile128(inputs["ln2_b"])
    bpb = tile128(inputs["b_proj"])
    b2b = tile128(inputs["b2"])
    b1c = np.ascontiguousarray(np.asarray(inputs["b1"], dtype=f).reshape(KH, 128).T)

    # exp maskbias: [128, 512]; cols 0:256 = keys 0:128 (allow q >= s),
    # cols 256:512 = keys 128:256 (allow q >= 128 + s)
    s_idx = np.arange(128)[:, None]
    q_idx = np.arange(T)[None, :]
    mb0 = np.where(q_idx >= s_idx, EXP_ALLOW, EXP_MASKED)
    mb1 = np.where(q_idx >= 128 + s_idx, EXP_ALLOW, EXP_MASKED)
    mbias = np.ascontiguousarray(np.concatenate([mb0, mb1], axis=1).astype(f))
    import ml_dtypes

    identb = np.ascontiguousarray(np.eye(128, dtype=ml_dtypes.bfloat16))
    ones1r = np.ones((1, 128), dtype=ml_dtypes.bfloat16)
    onescol = np.ones((128, 1), dtype=ml_dtypes.bfloat16)

    flags = (
        bool(not np.all(np.asarray(inputs["ln1_g"]) == 1.0)),
        bool(np.any(np.asarray(inputs["ln1_b"]))),
        bool(not np.all(np.asarray(inputs["ln2_g"]) == 1.0)),
        bool(np.any(np.asarray(inputs["ln2_b"]))),
        bool(np.any(np.asarray(inputs["b_proj"]))),
        bool(np.any(np.asarray(inputs["b1"]))),
        bool(np.any(np.asarray(inputs["b2"]))),
    )
    shared = dict(
        wq=_to_bf16(wq_full), wk=_to_bf16(wk_full), wv=_to_bf16(wv_full),
        wp=_to_bf16(wp_), w1=_to_bf16(w1_), w2=_to_bf16(w2_),
        g1=g1, b1ln=b1ln, g2=g2, b2ln=b2ln, bpb=bpb, b2b=b2b, b1c=b1c,
        mb=mbias, identb=identb, ones1r=ones1r, onescol=onescol,
    )
    in_maps = []
    for i in range(NCORES):
        m = dict(shared)
        m["x"] = np.ascontiguousarray(x[i * BL : (i + 1) * BL])
        in_maps.append(m)
    return in_maps, flags


_NC_CACHE = {}


def _get_program(flags):
    if flags not in _NC_CACHE:
        _NC_CACHE[flags] = build_program(*flags)
    return _NC_CACHE[flags]


def run(inputs, **spmd_kwargs):
    from concourse.bass_utils import run_bass_kernel_spmd

    in_maps, flags = _host_prep(inputs)
    nc = _get_program(flags)
    bkr = run_bass_kernel_spmd(nc, in_maps, list(range(NCORES)), **spmd_kwargs)
    outs = [bkr.results[i]["out"] for i in range(NCORES)]
    return np.concatenate(outs, axis=0).astype(np.float32), bkr


def kernel(**inputs):
    full, _ = run(inputs)
    return full


# revision 7
# speedup vs baseline: 2.8966x; 1.1006x over previous
"""Trainium2 Bass kernel for a dense transformer block (B=128, T=256, C=384, H=6).

Sharding: data-parallel over batch across 8 NeuronCores (16 batches/core),
identical SPMD program per core, no collectives.

Design (v4):
  - Attention computed in transposed orientation: scores are produced as
    S^T[key, query] directly (lhsT=k, rhs=q), so the softmax weights feed the
    attention matmul (lhsT=v, rhs=expS^T) with NO per-tile PE transposes and
    NO scalar-engine copies of the weights.
  - Softmax: exp + causal mask fused into ONE DVE op per score tile via an
    int16 Schraudolph (bf16 shares f32's 8-bit exponent):
    e = bitcast_bf16(int16(rint(s * 2^7/ln2 + maskbias))), maskbias holding
    127*2^7 for allowed entries and 40*2^7 (-> ~1e-26) for masked ones.
    End-to-end rel err ~9e-3 vs the 2e-2 budget.
  - Softmax denominators via ones-column matmuls on the PE (sum over keys is
    a partition-dim reduction); raw rowsums are PE-broadcast across
    partitions (tiny K=1 matmul), reciprocal runs partition-parallel on the
    broadcast (DVE reciprocal costs ~4 cyc/free-elem; approx_fast ~5x less),
    and the normalize multiply doubles as the PSUM->SBUF move of the
    attention output.
  - All matmuls bf16 (weights converted host-side; activations converted in
    the PSUM->SBUF copies). fp32 residual spine.
  - LayerNorm: stats on DVE (bn_stats), apply as x*rstd + (-mu*rstd) on the
    ACT engine (1-bucket affine table, same class as Copy). Relu on ACT.
  - Head pairs packed: score matmuls row-tiled (two K=64 heads concurrent in
    row groups 0-1/2-3), attention + broadcast matmuls col-tiled to stack the
    two heads into partitions 0:64 / 64:128 of one PSUM bank.
  - Software-pipelined emission: the in-order engine queues are laid out so
    no engine head-of-line-blocks: next pair's DMA+LN1 is emitted before the
    current pair's MLP; attention units pipeline scores(u+1) ahead of
    attn(u), and bcast/recip/normalize(u-1) behind attn(u).
"""

import numpy as np

import concourse.bass as bass
import concourse.mybir as mybir
from concourse import bacc
from concourse.tile import TileContext
from contextlib import ExitStack

B, T, C = 128, 256, 384
H, D = 6, 64
FF = 4 * C
NCORES = 8
BL = B // NCORES  # 16
NPAIR = BL // 2  # 8
KC = C // 128  # 3
KH = FF // 128  # 12
EPS = 1e-5
F32 = mybir.dt.float32
BF16 = mybir.dt.bfloat16
I32 = mybir.dt.int32
I16 = mybir.dt.int16
ALU = mybir.AluOpType
AF = mybir.ActivationFunctionType

EXP_S = float(2**7 / np.log(2.0))
EXP_ALLOW = float(127 * 2**7)
EXP_MASKED = float(40 * 2**7)
SQRT_MAGIC = 0x1FBD1DF5


def build_program(use_g1, use_b1ln, use_g2, use_b2ln, use_bp, use_b1, use_b2):
    nc = bacc.Bacc(None)
    x = nc.declare_dram_parameter("x", [BL, T, C], F32, isOutput=False)
    wq = nc.declare_dram_parameter("wq", [C, C], BF16, isOutput=False)
    wk = nc.declare_dram_parameter("wk", [C, C], BF16, isOutput=False)
    wv = nc.declare_dram_parameter("wv", [C, C], BF16, isOutput=False)
    wp = nc.declare_dram_parameter("wp", [C, C], BF16, isOutput=False)
    w1 = nc.declare_dram_parameter("w1", [C, FF], BF16, isOutput=False)
    w2 = nc.declare_dram_parameter("w2", [FF, C], BF16, isOutput=False)
    g1 = nc.declare_dram_parameter("g1", [128, C], F32, isOutput=False)
    b1ln = nc.declare_dram_parameter("b1ln", [128, C], F32, isOutput=False)
    g2 = nc.declare_dram_parameter("g2", [128, C], F32, isOutput=False)
    b2ln = nc.declare_dram_parameter("b2ln", [128, C], F32, isOutput=False)
    bpb = nc.declare_dram_parameter("bpb", [128, C], F32, isOutput=False)
    b2b = nc.declare_dram_parameter("b2b", [128, C], F32, isOutput=False)
    b1c = nc.declare_dram_parameter("b1c", [128, KH], F32, isOutput=False)
    mb = nc.declare_dram_parameter("mb", [128, 2 * T], F32, isOutput=False)
    identb = nc.declare_dram_parameter("identb", [128, 128], BF16, isOutput=False)
    ones1r = nc.declare_dram_parameter("ones1r", [1, 128], BF16, isOutput=False)
    onescol = nc.declare_dram_parameter("onescol", [128, 1], BF16, isOutput=False)
    out = nc.declare_dram_parameter("out", [BL, T, C], F32, isOutput=True)

    with TileContext(nc) as tc, ExitStack() as ctx:
        wts = ctx.enter_context(tc.tile_pool(name="wts", bufs=1))
        sb = ctx.enter_context(tc.tile_pool(name="sb", bufs=1))
        st = ctx.enter_context(tc.tile_pool(name="st", bufs=4))
        ps = ctx.enter_context(tc.tile_pool(name="ps", bufs=8, space="PSUM"))

        def load_chunks(dram, n, width, tagp, dt):
            tiles = []
            for k in range(n):
                t_ = wts.tile([128, width], dt, name=f"{tagp}{k}", tag=f"{tagp}{k}")
                nc.sync.dma_start(out=t_, in_=dram[k * 128 : (k + 1) * 128, :])
                tiles.append(t_)
            return tiles

        wq_sb = load_chunks(wq, KC, C, "wq", BF16)
        wk_sb = load_chunks(wk, KC, C, "wk", BF16)
        wv_sb = load_chunks(wv, KC, C, "wv", BF16)
        wp_sb = load_chunks(wp, KC, C, "wp", BF16)
        w1_sb = load_chunks(w1, KC, FF, "w1", BF16)
        w2_sb = load_chunks(w2, KH, C, "w2", BF16)

        def load_one(dram, shape, tag, dt=F32):
            t_ = wts.tile(shape, dt, name=tag, tag=tag)
            nc.sync.dma_start(out=t_, in_=dram[:, :])
            return t_

        g1_sb = load_one(g1, [128, C], "g1") if use_g1 else None
        b1ln_sb = load_one(b1ln, [128, C], "b1ln") if use_b1ln else None
        g2_sb = load_one(g2, [128, C], "g2") if use_g2 else None
        b2ln_sb = load_one(b2ln, [128, C], "b2ln") if use_b2ln else None
        bpb_sb = load_one(bpb, [128, C], "bpb") if use_bp else None
        b2b_sb = load_one(b2b, [128, C], "b2b") if use_b2 else None
        b1c_sb = load_one(b1c, [128, KH], "b1c") if use_b1 else None
        mb_sb = load_one(mb, [128, 2 * T], "mb")
        id_sb = load_one(identb, [128, 128], "identb", BF16)
        ones1r_sb = load_one(ones1r, [1, 128], "ones1r", BF16)
        onescol_sb = load_one(onescol, [128, 1], "onescol", BF16)

        def batched_rstd(mv8):
            """[128,8] interleaved (mean,var) x4 -> rstd4 [128,4]."""
            mv_v = mv8.rearrange("p (i two) -> p i two", two=2)
            var4 = mv_v[:, :, 1]
            vpe = st.tile([128, 4], F32, name="vpe", tag="vpe")
            nc.vector.tensor_scalar(
                out=vpe, in0=var4, scalar1=EPS, scalar2=None, op0=ALU.add
            )
            s0h = st.tile([128, 4], I32, name="s0h", tag="s0h")
            nc.vector.tensor_scalar(
                out=s0h, in0=vpe.bitcast(I32), scalar1=1, scalar2=None,
                op0=ALU.logical_shift_right,
            )
            s0i = st.tile([128, 4], I32, name="s0i", tag="s0i")
            nc.vector.tensor_scalar(
                out=s0i, in0=s0h, scalar1=SQRT_MAGIC, scalar2=None, op0=ALU.add
            )
            cur = s0i.bitcast(F32)
            # one Heron step on sqrt(v): seed err ~3.4% -> ~6e-4, plenty for
            # a bf16 downstream; chain LATENCY matters (PE idles behind it)
            for hi in range(1):
                r_ = st.tile([128, 4], F32, name=f"hr{hi}", tag=f"hr{hi}")
                nc.vector.reciprocal(r_, cur)
                t_ = st.tile([128, 4], F32, name=f"ht{hi}", tag=f"ht{hi}")
                nc.vector.tensor_mul(t_, vpe, r_)
                s_ = st.tile([128, 4], F32, name=f"hs{hi}", tag=f"hs{hi}")
                nc.vector.tensor_add(s_, t_, cur)
                sh = st.tile([128, 4], F32, name=f"hh{hi}", tag=f"hh{hi}")
                nc.vector.tensor_scalar_mul(sh, s_, 0.5)
                cur = sh
            rstd4 = st.tile([128, 4], F32, name="rstd4", tag="rstd4")
            nc.vector.reciprocal(rstd4, cur)
            return rstd4

        def layernorm4(dsts, srcs, g_sb, b_sb):
            mv8 = st.tile([128, 8], F32, name="mv8", tag="mv8")
            for i in range(4):
                stats = st.tile([128, 6], F32, name="lst", tag="lst")
                nc.vector.bn_stats(stats, srcs[i])
                nc.vector.bn_aggr(mv8[:, 2 * i : 2 * i + 2], stats)
            rstd4 = batched_rstd(mv8)
            # (x - mu) * rstd == x * rstd + (-mu * rstd): affine -> ACT
            mv_v = mv8.rearrange("p (i two) -> p i two", two=2)
            negmr = st.tile([128, 4], F32, name="negmr", tag="negmr")
            nc.vector.scalar_tensor_tensor(
                out=negmr, in0=mv_v[:, :, 0], scalar=-1.0, in1=rstd4,
                op0=ALU.mult, op1=ALU.mult,
            )
            for i in range(4):
                nc.scalar.activation(
                    out=dsts[i], in_=srcs[i], func=AF.Identity,
                    bias=negmr[:, i : i + 1], scale=rstd4[:, i : i + 1],
                )
                if g_sb is not None:
                    nc.vector.tensor_mul(dsts[i], dsts[i], g_sb)
                if b_sb is not None:
                    nc.vector.tensor_add(dsts[i], dsts[i], b_sb)

        def transpose_into(dstT, src, i):
            for c in range(KC):
                pt = ps.tile([128, 128], BF16, name="pa", tag="pa")
                nc.tensor.transpose(pt, src[:, c * 128 : (c + 1) * 128], id_sb)
                nc.any.tensor_copy(dstT[c][:, i * 128 : (i + 1) * 128], pt)

        def stage1_ln(p):
            """DMA x, LN1 -> ht_ (bf16). Returns pair state dict."""
            bs = [2 * p, 2 * p, 2 * p + 1, 2 * p + 1]
            tch = [0, 1, 0, 1]
            S = {"bs": bs, "tch": tch}
            S["xt"] = [
                sb.tile([128, C], F32, name=f"xt{i}", tag=f"xt{i}", bufs=2)
                for i in range(4)
            ]
            for i in range(4):
                nc.sync.dma_start(
                    out=S["xt"][i],
                    in_=x[bs[i], tch[i] * 128 : (tch[i] + 1) * 128, :],
                )
            S["ht"] = [
                sb.tile([128, C], BF16, name=f"h{i}", tag=f"h{i}", bufs=2)
                for i in range(4)
            ]
            layernorm4(S["ht"], S["xt"], g1_sb, b1ln_sb)
            return S

        def stage1_t(S):
            S["hT"] = [
                sb.tile([128, 2 * T], BF16, name=f"hT{c}", tag=f"hT{c}", bufs=2)
                for c in range(KC)
            ]
            for i in range(4):
                transpose_into(S["hT"], S["ht"][i], i)

        def stage2(S):
            hT = S["hT"]
            S["qT"] = [
                sb.tile([128, 2 * T], BF16, name=f"qT{m}", tag=f"qT{m}", bufs=2)
                for m in range(KC)
            ]
            S["kT"] = [
                sb.tile([128, 2 * T], BF16, name=f"kT{m}", tag=f"kT{m}", bufs=2)
                for m in range(KC)
            ]
            for m in range(KC):
                pq = ps.tile([128, 2 * T], F32, name="pa", tag="pa")
                for k in range(KC):
                    nc.tensor.matmul(
                        pq, wq_sb[k][:, m * 128 : (m + 1) * 128], hT[k],
                        start=(k == 0), stop=(k == KC - 1),
                    )
                nc.scalar.copy(S["qT"][m], pq)
                pk = ps.tile([128, 2 * T], F32, name="pa", tag="pa")
                for k in range(KC):
                    nc.tensor.matmul(
                        pk, wk_sb[k][:, m * 128 : (m + 1) * 128], hT[k],
                        start=(k == 0), stop=(k == KC - 1),
                    )
                nc.scalar.copy(S["kT"][m], pk)
            S["vt"] = [
                sb.tile([128, C], BF16, name=f"v{i}", tag=f"v{i}", bufs=2)
                for i in range(4)
            ]
            for i in range(4):
                pv = ps.tile([128, C], F32, name="pa", tag="pa")
                for k in range(KC):
                    nc.tensor.matmul(
                        pv, hT[k][:, i * 128 : (i + 1) * 128], wv_sb[k],
                        start=(k == 0), stop=(k == KC - 1),
                    )
                nc.any.tensor_copy(S["vt"][i], pv)

        def attention(S):
            """Software-pipelined: scores/exp one unit ahead of the attn
            matmuls; bcast/recip/normalize one unit behind."""
            qT, kT, vt = S["qT"], S["kT"], S["vt"]
            S["acT"] = [
                sb.tile([128, 2 * T], BF16, name=f"acT{c}", tag=f"acT{c}",
                        bufs=2)
                for c in range(KC)
            ]
            units = [(ib, ch) for ib in range(2) for ch in range(KC)]
            ust = [dict() for _ in units]

            def phase_a(u):
                ib, ch = units[u]
                q0 = ib * T
                eT = []
                for par in range(2):
                    off = par * 64
                    pS = ps.tile([128, 2 * T], F32, name="pa", tag="pa")
                    for kc in range(2):
                        nc.tensor.matmul(
                            pS[:, kc * T : (kc + 1) * T],
                            kT[ch][
                                off : off + 64,
                                q0 + kc * 128 : q0 + (kc + 1) * 128,
                            ],
                            qT[ch][off : off + 64, q0 : q0 + T],
                            start=True, stop=True,
                            tile_position=(off, 0),
                        )
                    e_ = sb.tile(
                        [128, 2 * T], I16,
                        name=f"eT{par}", tag=f"eT{par}", bufs=3,
                    )
                    nc.vector.scalar_tensor_tensor(
                        out=e_, in0=pS, scalar=EXP_S, in1=mb_sb,
                        op0=ALU.mult, op1=ALU.add,
                    )
                    eT.append(e_.bitcast(BF16))
                ust[u]["eT"] = eT

            def phase_c(u):
                ib, ch = units[u]
                eT = ust[u]["eT"]
                pA = ps.tile([128, T], F32, name="pa", tag="pa")
                rs = ps.tile([1, 2 * T], F32, name="pa", tag="pa")
                for par in range(2):
                    off = par * 64
                    hh = 2 * ch + par
                    for kc in range(2):
                        nc.tensor.matmul(
                            pA[off : off + 64, :],
                            vt[ib * 2 + kc][:, hh * 64 : hh * 64 + 64],
                            eT[par][:, kc * T : (kc + 1) * T],
                            start=(kc == 0), stop=(kc == 1),
                            tile_position=(0, off),
                        )
                    for kc in range(2):
                        nc.tensor.matmul(
                            rs[0:1, par * T : (par + 1) * T],
                            onescol_sb,
                            eT[par][:, kc * T : (kc + 1) * T],
                            start=(kc == 0), stop=(kc == 1),
                        )
                rsS = sb.tile([1, 2 * T], BF16, name="rsS", tag="rsS", bufs=3)
                nc.scalar.copy(rsS, rs)
                ust[u]["pA"], ust[u]["rsS"] = pA, rsS

            def phase_b(u):
                ib, ch = units[u]
                q0 = ib * T
                pA, rsS = ust[u]["pA"], ust[u]["rsS"]
                rsB = ps.tile([128, T], F32, name="pa", tag="pa")
                for par in range(2):
                    off = par * 64
                    nc.tensor.matmul(
                        rsB[off : off + 64, :],
                        ones1r_sb[0:1, 0:64],
                        rsS[0:1, par * T : (par + 1) * T],
                        start=True, stop=True,
                        tile_position=(0, off),
                    )
                rrS = sb.tile([128, T], F32, name="rrS", tag="rrS", bufs=3)
                nc.vector.reciprocal_approx_fast(out=rrS, in_=rsB)
                nc.vector.tensor_mul(S["acT"][ch][:, q0 : q0 + T], pA, rrS)

            phase_a(0)
            for u in range(len(units)):
                if u + 1 < len(units):
                    phase_a(u + 1)
                phase_c(u)
                if u >= 1:
                    phase_b(u - 1)
            phase_b(len(units) - 1)

        def proj_resid(S):
            S["yt"] = [
                sb.tile([128, C], F32, name=f"y{i}", tag=f"y{i}", bufs=2)
                for i in range(4)
            ]
            for i in range(4):
                pP = ps.tile([128, C], F32, name="pa", tag="pa")
                for k in range(KC):
                    nc.tensor.matmul(
                        pP, S["acT"][k][:, i * 128 : (i + 1) * 128], wp_sb[k],
                        start=(k == 0), stop=(k == KC - 1),
                    )
                nc.vector.tensor_add(S["yt"][i], pP, S["xt"][i])
                if bpb_sb is not None:
                    nc.vector.tensor_add(S["yt"][i], S["yt"][i], bpb_sb)

        def ln2_t(S):
            S["h2T"] = [
                sb.tile([128, 2 * T], BF16, name=f"h2T{c}", tag=f"h2T{c}",
                        bufs=2)
                for c in range(KC)
            ]
            h2_ = [
                sb.tile([128, C], BF16, name=f"h2{i}", tag=f"h2{i}", bufs=2)
                for i in range(4)
            ]
            layernorm4(h2_, S["yt"], g2_sb, b2ln_sb)
            for i in range(4):
                transpose_into(S["h2T"], h2_[i], i)

        def mlp_store(S):
            m1r = [
                sb.tile([128, 2 * T], BF16, name=f"m1r{m}", tag=f"m1r{m}",
                        bufs=2)
                for m in range(KH)
            ]
            for m in range(KH):
                pM = ps.tile([128, 2 * T], F32, name="pa", tag="pa")
                for k in range(KC):
                    nc.tensor.matmul(
                        pM, w1_sb[k][:, m * 128 : (m + 1) * 128], S["h2T"][k],
                        start=(k == 0), stop=(k == KC - 1),
                    )
                nc.scalar.activation(
                    out=m1r[m], in_=pM, func=AF.Relu,
                    bias=(b1c_sb[:, m : m + 1] if use_b1 else 0.0),
                    scale=1.0,
                )
            for i in range(4):
                psY = ps.tile([128, C], F32, name="pa", tag="pa")
                for m in range(KH):
                    nc.tensor.matmul(
                        psY, m1r[m][:, i * 128 : (i + 1) * 128], w2_sb[m],
                        start=(m == 0), stop=(m == KH - 1),
                    )
                ot = sb.tile([128, C], F32, name=f"ot{i}", tag=f"ot{i}", bufs=2)
                nc.vector.tensor_add(ot, psY, S["yt"][i])
                if b2b_sb is not None:
                    nc.vector.tensor_add(ot, ot, b2b_sb)
                nc.sync.dma_start(
                    out=out[
                        S["bs"][i], S["tch"][i] * 128 : (S["tch"][i] + 1) * 128, :
                    ],
                    in_=ot,
                )

        # --- software-pipelined pair loop ---
        # Emission order is engine-queue order; lay the queues out so the
        # serial LN chains (DVE) hide under next-pair PE work:
        #   attention(p) | LN1(p+1) chain | proj(p) | T1(p+1) | QKV(p+1)
        #   | LN2(p)+T5(p) | MLP(p) | attention(p+1) ...
        S = stage1_ln(0)
        stage1_t(S)
        stage2(S)
        nextS = None
        for p in range(NPAIR):
            attention(S)
            if p + 1 < NPAIR:
                nextS = stage1_ln(p + 1)
            proj_resid(S)
            if p + 1 < NPAIR:
                stage1_t(nextS)
                stage2(nextS)
            ln2_t(S)
            mlp_store(S)
            S = nextS
    nc.compile()
    return nc


def _to_bf16(a):
    import ml_dtypes

    return np.ascontiguousarray(a.astype(ml_dtypes.bfloat16))


def _host_prep(inputs):
    f = np.float32
    x = np.ascontiguousarray(inputs["x"], dtype=f)
    wq_full = (
        np.asarray(inputs["wq"], dtype=f).transpose(1, 0, 2).reshape(C, C)
        * (C ** -0.5)
    )
    wk_full = np.asarray(inputs["wk"], dtype=f).transpose(1, 0, 2).reshape(C, C)
    wv_full = np.asarray(inputs["wv"], dtype=f).transpose(1, 0, 2).reshape(C, C)
    wp_ = np.asarray(inputs["w_proj"], dtype=f)
    w1_ = np.asarray(inputs["w1"], dtype=f)
    w2_ = np.asarray(inputs["w2"], dtype=f)
    tile128 = lambda v: np.ascontiguousarray(
        np.broadcast_to(np.asarray(v, dtype=f), (128, C))
    )
    g1 = tile128(inputs["ln1_g"])
    b1ln = tile128(inputs["ln1_b"])
    g2 = tile128(inputs["ln2_g"])
    b2ln = tile128(inputs["ln2_b"])
    bpb = tile128(inputs["b_proj"])
    b2b = tile128(inputs["b2"])
    b1c = np.ascontiguousarray(np.asarray(inputs["b1"], dtype=f).reshape(KH, 128).T)

    # exp maskbias: [128, 512]; cols 0:256 = keys 0:128 (allow q >= s),
    # cols 256:512 = keys 128:256 (allow q >= 128 + s)
    s_idx = np.arange(128)[:, None]
    q_idx = np.arange(T)[None, :]
    mb0 = np.where(q_idx >= s_idx, EXP_ALLOW, EXP_MASKED)
    mb1 = np.where(q_idx >= 128 + s_idx, EXP_ALLOW, EXP_MASKED)
    mbias = np.ascontiguousarray(np.concatenate([mb0, mb1], axis=1).astype(f))
    import ml_dtypes

    identb = np.ascontiguousarray(np.eye(128, dtype=ml_dtypes.bfloat16))
    ones1r = np.ones((1, 128), dtype=ml_dtypes.bfloat16)
    onescol = np.ones((128, 1), dtype=ml_dtypes.bfloat16)

    flags = (
        bool(not np.all(np.asarray(inputs["ln1_g"]) == 1.0)),
        bool(np.any(np.asarray(inputs["ln1_b"]))),
        bool(not np.all(np.asarray(inputs["ln2_g"]) == 1.0)),
        bool(np.any(np.asarray(inputs["ln2_b"]))),
        bool(np.any(np.asarray(inputs["b_proj"]))),
        bool(np.any(np.asarray(inputs["b1"]))),
        bool(np.any(np.asarray(inputs["b2"]))),
    )
    shared = dict(
        wq=_to_bf16(wq_full), wk=_to_bf16(wk_full), wv=_to_bf16(wv_full),
        wp=_to_bf16(wp_), w1=_to_bf16(w1_), w2=_to_bf16(w2_),
        g1=g1, b1ln=b1ln, g2=g2, b2ln=b2ln, bpb=bpb, b2b=b2b, b1c=b1c,
        mb=mbias, identb=identb, ones1r=ones1r, onescol=onescol,
    )
    in_maps = []
    for i in range(NCORES):
        m = dict(shared)
        m["x"] = np.ascontiguousarray(x[i * BL : (i + 1) * BL])
        in_maps.append(m)
    return in_maps, flags


_NC_CACHE = {}


def _get_program(flags):
    if flags not in _NC_CACHE:
        _NC_CACHE[flags] = build_program(*flags)
    return _NC_CACHE[flags]


def run(inputs, **spmd_kwargs):
    from concourse.bass_utils import run_bass_kernel_spmd

    in_maps, flags = _host_prep(inputs)
    nc = _get_program(flags)
    bkr = run_bass_kernel_spmd(nc, in_maps, list(range(NCORES)), **spmd_kwargs)
    outs = [bkr.results[i]["out"] for i in range(NCORES)]
    return np.concatenate(outs, axis=0).astype(np.float32), bkr


def kernel(**inputs):
    full, _ = run(inputs)
    return full


# revision 8
# speedup vs baseline: 2.9106x; 1.0048x over previous
"""Trainium2 Bass kernel for a dense transformer block (B=128, T=256, C=384, H=6).

Sharding: data-parallel over batch across 8 NeuronCores (16 batches/core),
identical SPMD program per core, no collectives.

Design (v4):
  - Attention computed in transposed orientation: scores are produced as
    S^T[key, query] directly (lhsT=k, rhs=q), so the softmax weights feed the
    attention matmul (lhsT=v, rhs=expS^T) with NO per-tile PE transposes and
    NO scalar-engine copies of the weights.
  - Softmax: exp + causal mask fused into ONE DVE op per score tile via an
    int16 Schraudolph (bf16 shares f32's 8-bit exponent):
    e = bitcast_bf16(int16(rint(s * 2^7/ln2 + maskbias))), maskbias holding
    127*2^7 for allowed entries and 40*2^7 (-> ~1e-26) for masked ones.
    End-to-end rel err ~9e-3 vs the 2e-2 budget.
  - Softmax denominators via ones-column matmuls on the PE (sum over keys is
    a partition-dim reduction); raw rowsums are PE-broadcast across
    partitions (tiny K=1 matmul), reciprocal runs partition-parallel on the
    broadcast (DVE reciprocal costs ~4 cyc/free-elem; approx_fast ~5x less),
    and the normalize multiply doubles as the PSUM->SBUF move of the
    attention output.
  - All matmuls bf16 (weights converted host-side; activations converted in
    the PSUM->SBUF copies). fp32 residual spine.
  - LayerNorm: stats on DVE (bn_stats), apply as x*rstd + (-mu*rstd) on the
    ACT engine (1-bucket affine table, same class as Copy). Relu on ACT.
  - Head pairs packed: score matmuls row-tiled (two K=64 heads concurrent in
    row groups 0-1/2-3), attention + broadcast matmuls col-tiled to stack the
    two heads into partitions 0:64 / 64:128 of one PSUM bank.
  - Software-pipelined emission: the in-order engine queues are laid out so
    no engine head-of-line-blocks: next pair's DMA+LN1 is emitted before the
    current pair's MLP; attention units pipeline scores(u+1) ahead of
    attn(u), and bcast/recip/normalize(u-1) behind attn(u).
"""

import numpy as np

import concourse.bass as bass
import concourse.mybir as mybir
from concourse import bacc
from concourse.tile import TileContext
from contextlib import ExitStack

B, T, C = 128, 256, 384
H, D = 6, 64
FF = 4 * C
NCORES = 8
BL = B // NCORES  # 16
NPAIR = BL // 2  # 8
KC = C // 128  # 3
KH = FF // 128  # 12
EPS = 1e-5
F32 = mybir.dt.float32
BF16 = mybir.dt.bfloat16
I32 = mybir.dt.int32
I16 = mybir.dt.int16
ALU = mybir.AluOpType
AF = mybir.ActivationFunctionType

EXP_S = float(2**7 / np.log(2.0))
EXP_ALLOW = float(127 * 2**7)
EXP_MASKED = float(40 * 2**7)
SQRT_MAGIC = 0x1FBD1DF5


def build_program(use_g1, use_b1ln, use_g2, use_b2ln, use_bp, use_b1, use_b2):
    nc = bacc.Bacc(None)
    x = nc.declare_dram_parameter("x", [BL, T, C], F32, isOutput=False)
    wq = nc.declare_dram_parameter("wq", [C, C], BF16, isOutput=False)
    wk = nc.declare_dram_parameter("wk", [C, C], BF16, isOutput=False)
    wv = nc.declare_dram_parameter("wv", [C, C], BF16, isOutput=False)
    wp = nc.declare_dram_parameter("wp", [C, C], BF16, isOutput=False)
    w1 = nc.declare_dram_parameter("w1", [C, FF], BF16, isOutput=False)
    w2 = nc.declare_dram_parameter("w2", [FF, C], BF16, isOutput=False)
    g1 = nc.declare_dram_parameter("g1", [128, C], F32, isOutput=False)
    b1ln = nc.declare_dram_parameter("b1ln", [128, C], F32, isOutput=False)
    g2 = nc.declare_dram_parameter("g2", [128, C], F32, isOutput=False)
    b2ln = nc.declare_dram_parameter("b2ln", [128, C], F32, isOutput=False)
    bpb = nc.declare_dram_parameter("bpb", [128, C], F32, isOutput=False)
    b2b = nc.declare_dram_parameter("b2b", [128, C], F32, isOutput=False)
    b1c = nc.declare_dram_parameter("b1c", [128, KH], F32, isOutput=False)
    mb = nc.declare_dram_parameter("mb", [128, T + 128], F32, isOutput=False)
    identb = nc.declare_dram_parameter("identb", [128, 128], BF16, isOutput=False)
    ones1r = nc.declare_dram_parameter("ones1r", [1, 128], BF16, isOutput=False)
    onescol = nc.declare_dram_parameter("onescol", [128, 1], BF16, isOutput=False)
    out = nc.declare_dram_parameter("out", [BL, T, C], F32, isOutput=True)

    with TileContext(nc) as tc, ExitStack() as ctx:
        wts = ctx.enter_context(tc.tile_pool(name="wts", bufs=1))
        sb = ctx.enter_context(tc.tile_pool(name="sb", bufs=1))
        st = ctx.enter_context(tc.tile_pool(name="st", bufs=4))
        ps = ctx.enter_context(tc.tile_pool(name="ps", bufs=8, space="PSUM"))

        def load_chunks(dram, n, width, tagp, dt):
            tiles = []
            for k in range(n):
                t_ = wts.tile([128, width], dt, name=f"{tagp}{k}", tag=f"{tagp}{k}")
                nc.sync.dma_start(out=t_, in_=dram[k * 128 : (k + 1) * 128, :])
                tiles.append(t_)
            return tiles

        wq_sb = load_chunks(wq, KC, C, "wq", BF16)
        wk_sb = load_chunks(wk, KC, C, "wk", BF16)
        wv_sb = load_chunks(wv, KC, C, "wv", BF16)
        wp_sb = load_chunks(wp, KC, C, "wp", BF16)
        w1_sb = load_chunks(w1, KC, FF, "w1", BF16)
        w2_sb = load_chunks(w2, KH, C, "w2", BF16)

        def load_one(dram, shape, tag, dt=F32):
            t_ = wts.tile(shape, dt, name=tag, tag=tag)
            nc.sync.dma_start(out=t_, in_=dram[:, :])
            return t_

        g1_sb = load_one(g1, [128, C], "g1") if use_g1 else None
        b1ln_sb = load_one(b1ln, [128, C], "b1ln") if use_b1ln else None
        g2_sb = load_one(g2, [128, C], "g2") if use_g2 else None
        b2ln_sb = load_one(b2ln, [128, C], "b2ln") if use_b2ln else None
        bpb_sb = load_one(bpb, [128, C], "bpb") if use_bp else None
        b2b_sb = load_one(b2b, [128, C], "b2b") if use_b2 else None
        b1c_sb = load_one(b1c, [128, KH], "b1c") if use_b1 else None
        mb_sb = load_one(mb, [128, T + 128], "mb")
        id_sb = load_one(identb, [128, 128], "identb", BF16)
        ones1r_sb = load_one(ones1r, [1, 128], "ones1r", BF16)
        onescol_sb = load_one(onescol, [128, 1], "onescol", BF16)

        def batched_rstd(mv8):
            """[128,8] interleaved (mean,var) x4 -> rstd4 [128,4]."""
            mv_v = mv8.rearrange("p (i two) -> p i two", two=2)
            var4 = mv_v[:, :, 1]
            vpe = st.tile([128, 4], F32, name="vpe", tag="vpe")
            nc.vector.tensor_scalar(
                out=vpe, in0=var4, scalar1=EPS, scalar2=None, op0=ALU.add
            )
            s0h = st.tile([128, 4], I32, name="s0h", tag="s0h")
            nc.vector.tensor_scalar(
                out=s0h, in0=vpe.bitcast(I32), scalar1=1, scalar2=None,
                op0=ALU.logical_shift_right,
            )
            s0i = st.tile([128, 4], I32, name="s0i", tag="s0i")
            nc.vector.tensor_scalar(
                out=s0i, in0=s0h, scalar1=SQRT_MAGIC, scalar2=None, op0=ALU.add
            )
            cur = s0i.bitcast(F32)
            # one Heron step on sqrt(v): seed err ~3.4% -> ~6e-4, plenty for
            # a bf16 downstream; chain LATENCY matters (PE idles behind it)
            for hi in range(1):
                r_ = st.tile([128, 4], F32, name=f"hr{hi}", tag=f"hr{hi}")
                nc.vector.reciprocal(r_, cur)
                t_ = st.tile([128, 4], F32, name=f"ht{hi}", tag=f"ht{hi}")
                nc.vector.tensor_mul(t_, vpe, r_)
                s_ = st.tile([128, 4], F32, name=f"hs{hi}", tag=f"hs{hi}")
                nc.vector.tensor_add(s_, t_, cur)
                sh = st.tile([128, 4], F32, name=f"hh{hi}", tag=f"hh{hi}")
                nc.vector.tensor_scalar_mul(sh, s_, 0.5)
                cur = sh
            rstd4 = st.tile([128, 4], F32, name="rstd4", tag="rstd4")
            nc.vector.reciprocal(rstd4, cur)
            return rstd4

        def layernorm4(dsts, srcs, g_sb, b_sb):
            mv8 = st.tile([128, 8], F32, name="mv8", tag="mv8")
            for i in range(4):
                stats = st.tile([128, 6], F32, name="lst", tag="lst")
                nc.vector.bn_stats(stats, srcs[i])
                nc.vector.bn_aggr(mv8[:, 2 * i : 2 * i + 2], stats)
            rstd4 = batched_rstd(mv8)
            # (x - mu) * rstd == x * rstd + (-mu * rstd): affine -> ACT
            mv_v = mv8.rearrange("p (i two) -> p i two", two=2)
            negmr = st.tile([128, 4], F32, name="negmr", tag="negmr")
            nc.vector.scalar_tensor_tensor(
                out=negmr, in0=mv_v[:, :, 0], scalar=-1.0, in1=rstd4,
                op0=ALU.mult, op1=ALU.mult,
            )
            for i in range(4):
                nc.scalar.activation(
                    out=dsts[i], in_=srcs[i], func=AF.Identity,
                    bias=negmr[:, i : i + 1], scale=rstd4[:, i : i + 1],
                )
                if g_sb is not None:
                    nc.vector.tensor_mul(dsts[i], dsts[i], g_sb)
                if b_sb is not None:
                    nc.vector.tensor_add(dsts[i], dsts[i], b_sb)

        def transpose_into(dstT, src, i):
            for c in range(KC):
                pt = ps.tile([128, 128], BF16, name="pa", tag="pa")
                nc.tensor.transpose(pt, src[:, c * 128 : (c + 1) * 128], id_sb)
                nc.any.tensor_copy(dstT[c][:, i * 128 : (i + 1) * 128], pt)

        def stage1_ln(p):
            """DMA x, LN1 -> ht_ (bf16). Returns pair state dict."""
            bs = [2 * p, 2 * p, 2 * p + 1, 2 * p + 1]
            tch = [0, 1, 0, 1]
            S = {"bs": bs, "tch": tch}
            S["xt"] = [
                sb.tile([128, C], F32, name=f"xt{i}", tag=f"xt{i}", bufs=2)
                for i in range(4)
            ]
            for i in range(4):
                nc.sync.dma_start(
                    out=S["xt"][i],
                    in_=x[bs[i], tch[i] * 128 : (tch[i] + 1) * 128, :],
                )
            S["ht"] = [
                sb.tile([128, C], BF16, name=f"h{i}", tag=f"h{i}", bufs=2)
                for i in range(4)
            ]
            layernorm4(S["ht"], S["xt"], g1_sb, b1ln_sb)
            return S

        def stage1_t(S):
            S["hT"] = [
                sb.tile([128, 2 * T], BF16, name=f"hT{c}", tag=f"hT{c}", bufs=2)
                for c in range(KC)
            ]
            for i in range(4):
                transpose_into(S["hT"], S["ht"][i], i)

        def stage2(S):
            hT = S["hT"]
            S["qT"] = [
                sb.tile([128, 2 * T], BF16, name=f"qT{m}", tag=f"qT{m}", bufs=2)
                for m in range(KC)
            ]
            S["kT"] = [
                sb.tile([128, 2 * T], BF16, name=f"kT{m}", tag=f"kT{m}", bufs=2)
                for m in range(KC)
            ]
            for m in range(KC):
                pq = ps.tile([128, 2 * T], F32, name="pa", tag="pa")
                for k in range(KC):
                    nc.tensor.matmul(
                        pq, wq_sb[k][:, m * 128 : (m + 1) * 128], hT[k],
                        start=(k == 0), stop=(k == KC - 1),
                    )
                nc.scalar.copy(S["qT"][m], pq)
                pk = ps.tile([128, 2 * T], F32, name="pa", tag="pa")
                for k in range(KC):
                    nc.tensor.matmul(
                        pk, wk_sb[k][:, m * 128 : (m + 1) * 128], hT[k],
                        start=(k == 0), stop=(k == KC - 1),
                    )
                nc.scalar.copy(S["kT"][m], pk)
            S["vt"] = [
                sb.tile([128, C], BF16, name=f"v{i}", tag=f"v{i}", bufs=2)
                for i in range(4)
            ]
            for i in range(4):
                pv = ps.tile([128, C], F32, name="pa", tag="pa")
                for k in range(KC):
                    nc.tensor.matmul(
                        pv, hT[k][:, i * 128 : (i + 1) * 128], wv_sb[k],
                        start=(k == 0), stop=(k == KC - 1),
                    )
                nc.any.tensor_copy(S["vt"][i], pv)

        def attention(S):
            """Software-pipelined: scores/exp one unit ahead of the attn
            matmuls; bcast/recip/normalize one unit behind."""
            qT, kT, vt = S["qT"], S["kT"], S["vt"]
            S["acT"] = [
                sb.tile([128, 2 * T], BF16, name=f"acT{c}", tag=f"acT{c}",
                        bufs=2)
                for c in range(KC)
            ]
            units = [(ib, ch) for ib in range(2) for ch in range(KC)]
            ust = [dict() for _ in units]

            def phase_a(u):
                ib, ch = units[u]
                q0 = ib * T
                eT = []
                for par in range(2):
                    off = par * 64
                    # kc0: keys 0:128 x all 256 queries; kc1: keys 128:256 x
                    # queries 128:256 only (queries <128 are fully masked)
                    pS = ps.tile([128, T + 128], F32, name="pa", tag="pa")
                    nc.tensor.matmul(
                        pS[:, 0:T],
                        kT[ch][off : off + 64, q0 : q0 + 128],
                        qT[ch][off : off + 64, q0 : q0 + T],
                        start=True, stop=True,
                        tile_position=(off, 0),
                    )
                    nc.tensor.matmul(
                        pS[:, T : T + 128],
                        kT[ch][off : off + 64, q0 + 128 : q0 + T],
                        qT[ch][off : off + 64, q0 + 128 : q0 + T],
                        start=True, stop=True,
                        tile_position=(off, 0),
                    )
                    e_ = sb.tile(
                        [128, T + 128], I16,
                        name=f"eT{par}", tag=f"eT{par}", bufs=3,
                    )
                    nc.vector.scalar_tensor_tensor(
                        out=e_, in0=pS, scalar=EXP_S, in1=mb_sb,
                        op0=ALU.mult, op1=ALU.add,
                    )
                    eT.append(e_.bitcast(BF16))
                ust[u]["eT"] = eT

            def phase_c(u):
                ib, ch = units[u]
                eT = ust[u]["eT"]
                pA = ps.tile([128, T], F32, name="pa", tag="pa")
                rs = ps.tile([1, 2 * T], F32, name="pa", tag="pa")
                for par in range(2):
                    off = par * 64
                    hh = 2 * ch + par
                    nc.tensor.matmul(
                        pA[off : off + 64, :],
                        vt[ib * 2][:, hh * 64 : hh * 64 + 64],
                        eT[par][:, 0:T],
                        start=True, stop=False,
                        tile_position=(0, off),
                    )
                    nc.tensor.matmul(
                        pA[off : off + 64, 128:T],
                        vt[ib * 2 + 1][:, hh * 64 : hh * 64 + 64],
                        eT[par][:, T : T + 128],
                        start=False, stop=True,
                        tile_position=(0, off),
                    )
                    nc.tensor.matmul(
                        rs[0:1, par * T : (par + 1) * T],
                        onescol_sb,
                        eT[par][:, 0:T],
                        start=True, stop=False,
                    )
                    nc.tensor.matmul(
                        rs[0:1, par * T + 128 : (par + 1) * T],
                        onescol_sb,
                        eT[par][:, T : T + 128],
                        start=False, stop=True,
                    )
                rsS = sb.tile([1, 2 * T], BF16, name="rsS", tag="rsS", bufs=3)
                nc.scalar.copy(rsS, rs)
                ust[u]["pA"], ust[u]["rsS"] = pA, rsS

            def phase_b(u):
                ib, ch = units[u]
                q0 = ib * T
                pA, rsS = ust[u]["pA"], ust[u]["rsS"]
                rsB = ps.tile([128, T], F32, name="pa", tag="pa")
                for par in range(2):
                    off = par * 64
                    nc.tensor.matmul(
                        rsB[off : off + 64, :],
                        ones1r_sb[0:1, 0:64],
                        rsS[0:1, par * T : (par + 1) * T],
                        start=True, stop=True,
                        tile_position=(0, off),
                    )
                rrS = sb.tile([128, T], F32, name="rrS", tag="rrS", bufs=3)
                nc.vector.reciprocal_approx_fast(out=rrS, in_=rsB)
                nc.vector.tensor_mul(S["acT"][ch][:, q0 : q0 + T], pA, rrS)

            phase_a(0)
            for u in range(len(units)):
                if u + 1 < len(units):
                    phase_a(u + 1)
                phase_c(u)
                if u >= 1:
                    phase_b(u - 1)
            phase_b(len(units) - 1)

        def proj_resid(S):
            S["yt"] = [
                sb.tile([128, C], F32, name=f"y{i}", tag=f"y{i}", bufs=2)
                for i in range(4)
            ]
            for i in range(4):
                pP = ps.tile([128, C], F32, name="pa", tag="pa")
                for k in range(KC):
                    nc.tensor.matmul(
                        pP, S["acT"][k][:, i * 128 : (i + 1) * 128], wp_sb[k],
                        start=(k == 0), stop=(k == KC - 1),
                    )
                nc.vector.tensor_add(S["yt"][i], pP, S["xt"][i])
                if bpb_sb is not None:
                    nc.vector.tensor_add(S["yt"][i], S["yt"][i], bpb_sb)

        def ln2_t(S):
            S["h2T"] = [
                sb.tile([128, 2 * T], BF16, name=f"h2T{c}", tag=f"h2T{c}",
                        bufs=2)
                for c in range(KC)
            ]
            h2_ = [
                sb.tile([128, C], BF16, name=f"h2{i}", tag=f"h2{i}", bufs=2)
                for i in range(4)
            ]
            layernorm4(h2_, S["yt"], g2_sb, b2ln_sb)
            for i in range(4):
                transpose_into(S["h2T"], h2_[i], i)

        def mlp_store(S):
            m1r = [
                sb.tile([128, 2 * T], BF16, name=f"m1r{m}", tag=f"m1r{m}",
                        bufs=2)
                for m in range(KH)
            ]
            for m in range(KH):
                pM = ps.tile([128, 2 * T], F32, name="pa", tag="pa")
                for k in range(KC):
                    nc.tensor.matmul(
                        pM, w1_sb[k][:, m * 128 : (m + 1) * 128], S["h2T"][k],
                        start=(k == 0), stop=(k == KC - 1),
                    )
                nc.scalar.activation(
                    out=m1r[m], in_=pM, func=AF.Relu,
                    bias=(b1c_sb[:, m : m + 1] if use_b1 else 0.0),
                    scale=1.0,
                )
            for i in range(4):
                psY = ps.tile([128, C], F32, name="pa", tag="pa")
                for m in range(KH):
                    nc.tensor.matmul(
                        psY, m1r[m][:, i * 128 : (i + 1) * 128], w2_sb[m],
                        start=(m == 0), stop=(m == KH - 1),
                    )
                ot = sb.tile([128, C], F32, name=f"ot{i}", tag=f"ot{i}", bufs=2)
                nc.vector.tensor_add(ot, psY, S["yt"][i])
                if b2b_sb is not None:
                    nc.vector.tensor_add(ot, ot, b2b_sb)
                nc.sync.dma_start(
                    out=out[
                        S["bs"][i], S["tch"][i] * 128 : (S["tch"][i] + 1) * 128, :
                    ],
                    in_=ot,
                )

        # --- software-pipelined pair loop ---
        # Emission order is engine-queue order; lay the queues out so the
        # serial LN chains (DVE) hide under next-pair PE work:
        #   attention(p) | LN1(p+1) chain | proj(p) | T1(p+1) | QKV(p+1)
        #   | LN2(p)+T5(p) | MLP(p) | attention(p+1) ...
        S = stage1_ln(0)
        stage1_t(S)
        stage2(S)
        nextS = None
        for p in range(NPAIR):
            attention(S)
            if p + 1 < NPAIR:
                nextS = stage1_ln(p + 1)
            proj_resid(S)
            if p + 1 < NPAIR:
                stage1_t(nextS)
                stage2(nextS)
            ln2_t(S)
            mlp_store(S)
            S = nextS
    nc.compile()
    return nc


def _to_bf16(a):
    import ml_dtypes

    return np.ascontiguousarray(a.astype(ml_dtypes.bfloat16))


def _host_prep(inputs):
    f = np.float32
    x = np.ascontiguousarray(inputs["x"], dtype=f)
    wq_full = (
        np.asarray(inputs["wq"], dtype=f).transpose(1, 0, 2).reshape(C, C)
        * (C ** -0.5)
    )
    wk_full = np.asarray(inputs["wk"], dtype=f).transpose(1, 0, 2).reshape(C, C)
    wv_full = np.asarray(inputs["wv"], dtype=f).transpose(1, 0, 2).reshape(C, C)
    wp_ = np.asarray(inputs["w_proj"], dtype=f)
    w1_ = np.asarray(inputs["w1"], dtype=f)
    w2_ = np.asarray(inputs["w2"], dtype=f)
    tile128 = lambda v: np.ascontiguousarray(
        np.broadcast_to(np.asarray(v, dtype=f), (128, C))
    )
    g1 = tile128(inputs["ln1_g"])
    b1ln = tile128(inputs["ln1_b"])
    g2 = tile128(inputs["ln2_g"])
    b2ln = tile128(inputs["ln2_b"])
    bpb = tile128(inputs["b_proj"])
    b2b = tile128(inputs["b2"])
    b1c = np.ascontiguousarray(np.asarray(inputs["b1"], dtype=f).reshape(KH, 128).T)

    # exp maskbias: [128, 512]; cols 0:256 = keys 0:128 (allow q >= s),
    # cols 256:512 = keys 128:256 (allow q >= 128 + s)
    s_idx = np.arange(128)[:, None]
    q_idx = np.arange(T)[None, :]
    q_hi = np.arange(128, T)[None, :]
    mb0 = np.where(q_idx >= s_idx, EXP_ALLOW, EXP_MASKED)
    mb1 = np.where(q_hi >= 128 + s_idx, EXP_ALLOW, EXP_MASKED)
    mbias = np.ascontiguousarray(np.concatenate([mb0, mb1], axis=1).astype(f))
    import ml_dtypes

    identb = np.ascontiguousarray(np.eye(128, dtype=ml_dtypes.bfloat16))
    ones1r = np.ones((1, 128), dtype=ml_dtypes.bfloat16)
    onescol = np.ones((128, 1), dtype=ml_dtypes.bfloat16)

    flags = (
        bool(not np.all(np.asarray(inputs["ln1_g"]) == 1.0)),
        bool(np.any(np.asarray(inputs["ln1_b"]))),
        bool(not np.all(np.asarray(inputs["ln2_g"]) == 1.0)),
        bool(np.any(np.asarray(inputs["ln2_b"]))),
        bool(np.any(np.asarray(inputs["b_proj"]))),
        bool(np.any(np.asarray(inputs["b1"]))),
        bool(np.any(np.asarray(inputs["b2"]))),
    )
    shared = dict(
        wq=_to_bf16(wq_full), wk=_to_bf16(wk_full), wv=_to_bf16(wv_full),
        wp=_to_bf16(wp_), w1=_to_bf16(w1_), w2=_to_bf16(w2_),
        g1=g1, b1ln=b1ln, g2=g2, b2ln=b2ln, bpb=bpb, b2b=b2b, b1c=b1c,
        mb=mbias, identb=identb, ones1r=ones1r, onescol=onescol,
    )
    in_maps = []
    for i in range(NCORES):
        m = dict(shared)
        m["x"] = np.ascontiguousarray(x[i * BL : (i + 1) * BL])
        in_maps.append(m)
    return in_maps, flags


_NC_CACHE = {}


def _get_program(flags):
    if flags not in _NC_CACHE:
        _NC_CACHE[flags] = build_program(*flags)
    return _NC_CACHE[flags]


def run(inputs, **spmd_kwargs):
    from concourse.bass_utils import run_bass_kernel_spmd

    in_maps, flags = _host_prep(inputs)
    nc = _get_program(flags)
    bkr = run_bass_kernel_spmd(nc, in_maps, list(range(NCORES)), **spmd_kwargs)
    outs = [bkr.results[i]["out"] for i in range(NCORES)]
    return np.concatenate(outs, axis=0).astype(np.float32), bkr


def kernel(**inputs):
    full, _ = run(inputs)
    return full
